# revision 32
# baseline (speedup 1.0000x reference)
"""Trainium2 Bass kernel for nn_DualAttentionLayer (dense dual-stream
transformer layer: 2x self-attention -> cross-attention -> gated merge ->
FFN, with layernorms).

Sharding: 8 cores = 4 batches x 2 streams. Core c handles batch c//2,
stream c%2 (0=body, 1=limb). Each core redundantly computes BOTH streams'
self-attention+LN stage (so no inter-core communication is needed), then
its own stream's cross-attention, gate, FFN and final norms.

v3: fp8 everywhere on the PE + engine rebalance.
 - All projections (QKV, out-proj, FFN w1/w2, gate) run fp8e4m3 with
   DoubleRow perf mode.  Weights pre-scaled x16 on the host; scale
   factors fold into eviction scales / softmax exp scale / LN betas.
 - Self-attn out-proj biases fold into the host-side residual streams.
 - FFN w2 bias folds into LN_b's fp32-path beta.
 - Scores stay bf16 (K=64 per head).  exp() writes fp8e5 u; softmax
   rowsums via a ones-column in V; per-head batched reciprocal (bf16)
   + gpsimd partition broadcast; normalize applies out-of-place into a
   fp8 ou tile that feeds the fp8 out-proj.
 - LayerNorm stats via bf16 ones-matmul; normalize interior in bf16
   (from the bf16 stats copy) even when a fp32 output is requested;
   squares for variance run on Pool.
 - PSUM eviction tiles widened to [128, 1024] (fewer, larger DVE/ACT
   evictions).
"""

import math
import numpy as np
from contextlib import ExitStack

import concourse.bacc as bacc
import concourse.bass as bass
import concourse.mybir as mybir
import concourse.tile as tile
from concourse.bass_utils import run_bass_kernel_spmd

dt = mybir.dt
AF = mybir.ActivationFunctionType
ALU = mybir.AluOpType
PM = mybir.MatmulPerfMode
BF16 = dt.np(dt.bfloat16)
F8NP = dt.np(dt.float8e4)

B, S, E, NH, D = 4, 1024, 512, 8, 64
HID = 4 * E
P = 128
KS = E // P          # 4 feature slabs of 128
MT = S // P          # 8 token m-tiles of 128
HKS = HID // P       # 16 hidden slabs
EPS = 1e-5
WS = 16.0            # host-side fp8 weight scale
C_EXP = 16.0         # softmax exp output scale (cancels in normalization)
LN_C = math.log(C_EXP)
VB = NH * 65 + 8     # v block stride per k-tile, padded to 528:
                     # dual-fp8 Ldweights needs pair stride % 16 == 0

F32 = dt.float32
BF = dt.bfloat16
F8 = dt.float8e4
F8U = dt.float8e5


def _build_nc(scale: float):
    nc = bacc.Bacc("TRN2", target_bir_lowering=False, debug=False,
                   num_devices=8)

    def din(name, shape, dty=F32):
        return nc.dram_tensor(name, shape, dty, kind="ExternalInput").ap()

    # activations (pre-transposed on host, feature-major [E, S])
    xo32 = din("xo32", [E, S])          # own residual = 16*(x+ob_eff), fp32
    xt16 = din("xt16", [E, S], BF)      # oth residual = 16*(x+ob_eff), bf16
    xo8 = din("xo8", [E, S], F8)        # own, fp8 (matmul rhs)
    xt8 = din("xt8", [E, S], F8)

    # attention weight sets: a = self-own, b = self-other, c = cross
    # all fp8 (x16)
    attw = {}
    for tag in ("a", "b", "c"):
        for m in ("qw", "kw", "vw", "ow"):
            attw[tag + m] = din(tag + m, [E, E], F8)
        attw[tag + "qb"] = din(tag + "qb", [E])     # 16*qb
    attw["cob"] = din("cob", [E])       # cross ob + vb@ow (unscaled)

    f1w = din("f1w", [E, HID], BF)      # bf16 (fp8 here costs too much err)
    f1b = din("f1b", [HID])
    f2w = din("f2w", [HID, E], F8)      # 16*w2
    f2b = din("f2b", [E])               # unused on device (folded in nbg2)
    gw = din("gw", [2 * E, 2], F8)
    gbd = din("gbd", [1, 1])            # gate_b[0] - gate_b[1]

    # norm params: a_own, a_oth (post-self-attn), b (post-gate), c (post-ffn)
    # "nb" has two betas: nbb2 = beta + f2b (for the fp32 residual copy)
    nrm = {}
    for tag in ("nao", "nat", "nb", "nc"):
        nrm[tag + "g"] = din(tag + "g", [E])
        nrm[tag + "b"] = din(tag + "b", [E])
    nrm["nbb2"] = din("nbb2", [E])

    out_t = nc.dram_tensor("outT", [E, S], F32, kind="ExternalOutput").ap()

    with TileKernel(nc, scale) as tk:
        tk.run(xo32, xt16, xo8, xt8, attw, f1w, f1b, f2w, gw, gbd,
               nrm, out_t)

    nc.finalize()
    return nc


class TileKernel:
    def __init__(self, nc, scale):
        self.nc = nc
        self.scale = float(scale)
        self.ctx = ExitStack()
        self.poolid = 0

    def __enter__(self):
        self.tc = self.ctx.enter_context(tile.TileContext(self.nc))
        return self

    def __exit__(self, *a):
        return self.ctx.__exit__(*a)

    # ---------- helpers ----------

    def load_vec(self, pool, dram_ap, n, name=None):
        """Load a [n*128] fp32 vector as [128, n] (slab per column)."""
        t = pool.tile([P, n], F32, tag=name)
        self.nc.sync.dma_start(
            t[:], dram_ap.rearrange("(s p) -> p s", p=P))
        return t

    def load_w8(self, pool, dram_ap, in_dim, out_dim, name=None, dty=F8):
        """Load weight [in,out] as [128, (in/128)*out] slab-major."""
        ks = in_dim // P
        t = pool.tile([P, ks * out_dim], dty, tag=name)
        self.nc.sync.dma_start(
            t[:].rearrange("p (s o) -> p s o", s=ks),
            dram_ap.rearrange("(s p) o -> p s o", p=P))
        return t

    def proj_bf(self, wsb, rhs16, in_dim, out_dim, evict, bufs=4):
        """bf16 Form-B projection: out^T = W^T @ x^T."""
        nc = self.nc
        self.poolid += 1
        with self.tc.tile_pool(name=f"pb{self.poolid}", bufs=bufs,
                               space="PSUM") as pp:
            nks = in_dim // P
            wr = wsb[:].rearrange("p (s o) -> p s o", s=nks)
            xr = rhs16[:].rearrange("p (s t) -> p s t", s=nks)
            for ms in range(out_dim // P):
                ps = pp.tile([P, S], F32, tag="proj", name="proj")
                for win in range(2):
                    o = ps[:, win * 512: win * 512 + 512]
                    for k in range(nks):
                        nc.tensor.matmul(
                            o, wr[:, k, ms * P: ms * P + P],
                            xr[:, k, win * 512: win * 512 + 512],
                            start=(k == 0), stop=(k == nks - 1))
                evict(ps, ms)

    def proj_f8(self, wsb, rhs8, in_dim, out_dim, evict, bufs=4,
                halves=False):
        """out^T[out,tok] = (W^T @ x^T) with fp8 DoubleRow matmuls.
        evict(ps, ms) consumes a [128, S] fp32 PSUM tile (full token
        range); with halves=True, evict(ps, ms, half) gets 512-token
        halves as their accumulation chains finish."""
        nc = self.nc
        self.poolid += 1
        with self.tc.tile_pool(name=f"pp{self.poolid}", bufs=bufs,
                               space="PSUM") as pp:
            self._proj_f8(pp, wsb, rhs8, in_dim, out_dim, evict,
                          halves=halves)

    def _proj_f8(self, pp, wsb, rhs8, in_dim, out_dim, evict,
                 halves=False):
        nc = self.nc
        nks = in_dim // P
        npr = nks // 2
        wr = wsb[:].rearrange("p (s o) -> p s o", s=nks)
        xr = rhs8[:].rearrange("p (s t) -> p s t", s=nks)
        for ms in range(out_dim // P):
            ps = pp.tile([P, S], F32, tag="proj", name="proj")
            for win in range(S // 256):
                o = ps[:, win * 256: win * 256 + 256]
                toff = win * 256
                for kp in range(npr):
                    nc.tensor.matmul(
                        o,
                        wr[:, 2 * kp: 2 * kp + 2, ms * P: ms * P + P],
                        xr[:, 2 * kp: 2 * kp + 2, toff: toff + 256],
                        start=(kp == 0), stop=(kp == npr - 1),
                        perf_mode=PM.DoubleRow)
                if halves and win % 2 == 1:
                    evict(ps, ms, win // 2)
            if not halves:
                evict(ps, ms)

    # ---------- attention ----------

    def attention_core(self, name, q8, kv8, wq, wk, wv, qb, oup,
                       kv_first=False):
        """MHA core: fp8 projections, bf16 scores, fp8 exp, DoubleRow AV,
        streamed softmax normalization.  Returns the normalized per-head
        output ou8 (fp8, tile in caller pool `oup`, = 16x true o).
        The caller runs the fp8 out-projection separately
        (attention_finish) so the next attention's core can overlap this
        one's tail."""
        nc, tc = self.nc, self.tc
        ou = oup.tile([P, KS * S], F8, tag="ou", name="ou_" + name,
                      bufs=2)
        with ExitStack() as actx:
            ap = actx.enter_context(
                tc.tile_pool(name="attc_" + name, bufs=1))
            up = actx.enter_context(
                tc.tile_pool(name="attu_" + name, bufs=3))

            qt = ap.tile([P, KS * S], BF, tag="qT")
            kt = ap.tile([P, KS * S], BF, tag="kT")
            vt = ap.tile([P, MT * VB], F8, tag="vT")

            def ev_q(ps, ms):
                nc.vector.tensor_scalar(
                    qt[:, ms * S: ms * S + S],
                    ps[:], qb[:, ms: ms + 1], None, op0=ALU.add)

            def ev_k(ps, ms):
                nc.vector.tensor_copy(
                    kt[:, ms * S: ms * S + S], ps[:])

            self.poolid += 1
            with tc.tile_pool(name=f"attn_pp{self.poolid}", bufs=3,
                              space="PSUM") as pp, \
                 tc.tile_pool(name=f"attn_pv{self.poolid}", bufs=2,
                              space="PSUM") as pv:
                if kv_first:
                    # cross-attn: K/V inputs (the other stream's LN) are
                    # ready before Q's — emit them first
                    self._proj_f8(pp, wk, kv8, E, E, ev_k)
                    self._proj_f8(pp, wq, q8, E, E, ev_q)
                else:
                    self._proj_f8(pp, wq, q8, E, E, ev_q)
                    self._proj_f8(pp, wk, kv8, E, E, ev_k)

                # V: Form A (x^T as lhsT) -> token-major v [tok, feat],
                # strided into per-head 65-wide blocks, col 64 = 1.
                v4 = vt[:].rearrange("p (m c) -> p m c", m=MT)
                nc.gpsimd.memset(
                    v4[:, :, 0:NH * 65]
                    .rearrange("p m (h c) -> p m h c", h=NH)
                    [:, :, :, 64:65], 1.0)
                xr = kv8[:].rearrange("p (s t) -> p s t", s=KS)
                wvr = wv[:].rearrange("p (s o) -> p s o", s=KS)
                for mt in range(MT):
                    ps = pv.tile([P, 512], F32, tag="vproj", name="vproj")
                    for fw in range(2):
                        o = ps[:, fw * 256: fw * 256 + 256]
                        for kp in range(2):
                            nc.tensor.matmul(
                                o,
                                xr[:, 2 * kp: 2 * kp + 2, mt * P: mt * P + P],
                                wvr[:, 2 * kp: 2 * kp + 2,
                                    fw * 256: fw * 256 + 256],
                                start=(kp == 0), stop=(kp == 1),
                                perf_mode=PM.DoubleRow)
                    nc.scalar.copy(
                        v4[:, mt, 0:NH * 65]
                        .rearrange("p (h c) -> p h c", h=NH)[:, :, 0:64],
                        ps[:].rearrange("p (h d) -> p h d", h=NH))

            self.poolid += 1
            scav = ExitStack()
            sp = scav.enter_context(
                tc.tile_pool(name=f"attn_sc{self.poolid}", bufs=2,
                             space="PSUM"))
            avp = scav.enter_context(
                tc.tile_pool(name=f"attn_av{self.poolid}", bufs=4,
                             space="PSUM"))

            v4 = vt[:].rearrange("p (m c) -> p m c", m=MT)
            sexp = self.scale / (WS * WS)
            # head pairs outer: each pair's softmax rowsums are
            # reciprocal'd + broadcast + applied as soon as the pair is
            # done, overlapping the remaining pairs' scores/exp/AV.
            NT = 2
            for j in range(NH // 2):
                hs = (2 * j, 2 * j + 1)
                # per-pair staging across both qn halves: head h%2==0 in
                # partitions 0-63, h%2==1 in 64-127 (TensorTensor needs
                # matching operand base partitions)
                tmp = up.tile([P, S], BF, tag="avtmp", name="avtmp",
                              bufs=2)
                st = {h: up.tile([1, S], F32, tag="rstage",
                                 name=f"rstage{h % 2}", bufs=4)
                      for h in hs}
                for qn in range(NT):
                    av = {(h, w): avp.tile([65, 256], F32, tag="av",
                                           name=f"av{h}_{w}")
                          for h in hs for w in range(2)}
                    for wave in range(MT // 2):
                        sc = {h: sp.tile([P, 1024], F32, tag="sc",
                                         name=f"sc{h}") for h in hs}
                        for i in range(2):
                            mt = wave * 2 + i
                            for h in hs:
                                bp = (h % 2) * 64
                                sl = h // 2
                                nc.tensor.matmul(
                                    sc[h][:, i * 512: i * 512 + 512],
                                    kt[bp: bp + 64,
                                       sl * S + mt * P: sl * S + mt * P + P],
                                    qt[bp: bp + 64,
                                       sl * S + qn * 512: sl * S + qn * 512 + 512],
                                    start=True, stop=True)
                        ut = {h: up.tile([P, 1024], F8, tag="u",
                                         name=f"u{h}") for h in hs}
                        for h in hs:
                            nc.scalar.activation(
                                ut[h][:], sc[h][:], AF.Exp, scale=sexp,
                                bias=self.lnc_c[:, 0:1])
                        for h in hs:
                            utr = ut[h][:].rearrange("p (i t) -> p i t", i=2)
                            for win in range(2):
                                nc.tensor.matmul(
                                    av[h, win][:],
                                    v4[:, 2 * wave: 2 * wave + 2,
                                       h * 65: h * 65 + 65],
                                    utr[:, :, win * 256: win * 256 + 256],
                                    start=(wave == 0), stop=(wave == 3),
                                    perf_mode=PM.DoubleRow)
                    for h in hs:
                        bp = (h % 2) * 64
                        # unnormalized o^T and rowsum -> bf16 staging
                        for win in range(2):
                            qo = qn * 512 + win * 256
                            nc.vector.tensor_copy(
                                tmp[bp: bp + 64, qo: qo + 256],
                                av[h, win][0:64, :])
                            nc.vector.tensor_copy(
                                st[h][:, qo: qo + 256],
                                av[h, win][64:65, :])
                for h in hs:
                    bp = (h % 2) * 64
                    sl = h // 2
                    rr = up.tile([1, S], BF, tag="rrec", name="rrec")
                    with nc.allow_low_precision(
                            reason="1/rowsum to bf16 is plenty"):
                        nc.vector.reciprocal(rr[:], st[h][:])
                    # full-128 broadcast: HW ucode mishandles
                    # non-zero destination base partitions
                    rbt = ap.tile([P, S], BF, tag="rbh",
                                  name=f"rbh{h % 2}", bufs=4)
                    nc.gpsimd.partition_broadcast(rbt[:], rr[:])
                    nc.vector.tensor_tensor(
                        ou[bp: bp + 64, sl * S: sl * S + S],
                        tmp[bp: bp + 64, :], rbt[bp: bp + 64, :],
                        op=ALU.mult)
            scav.close()
        return ou

    def attention_finish(self, ou8, wo, evict_out, bufs=2):
        """fp8 out projection of a finished attention core."""
        self.proj_f8(wo, ou8, E, E, evict_out, bufs=bufs)

    # ---------- layernorm ----------

    def layer_norm(self, t32, gam, bet, out32, out8, out16=None,
                   out_dma=None, bf_in=False, bet32=None, hi_fi=False):
        """LN over features (partition axis) of t32 [128, KS*S].
        Stats come from a bf16 copy (ones-matmul over partitions); the
        normalize interior runs in bf16 (or fp32 when hi_fi, for the
        final output norm); the fp32 output (if requested) applies
        gamma/bet32 from the interior.  Optional fp8 side output;
        out_dma streams the fp32 output to DRAM per slab.
        bf_in: t32 is already bf16 (skip the cast)."""
        nc = self.nc
        if bet32 is None:
            bet32 = bet
        self.poolid += 1
        with self.tc.tile_pool(name=f"lnsb{self.poolid}", bufs=1) as lnp:
            if bf_in:
                t16 = t32
            else:
                t16 = lnp.tile([P, KS * S], BF, tag="ln_t16")
                for nt in range(2):
                    for k in range(KS):
                        sl = slice(k * S + nt * 512, k * S + nt * 512 + 512)
                        nc.vector.tensor_copy(t16[:, sl], t32[:, sl])
            mu = lnp.tile([1, S], F32, tag="ln_mu", name="ln_mu")
            var = lnp.tile([1, S], F32, tag="ln_row", name="ln_var",
                           bufs=2)
            self.poolid += 1
            with self.tc.tile_pool(name=f"lnp{self.poolid}", bufs=2,
                                   space="PSUM") as sp1:
                for nt in range(2):
                    pmu = sp1.tile([1, 512], F32, tag="ln_stat", name="pmu")
                    psq = sp1.tile([1, 512], F32, tag="ln_stat", name="psq")
                    for k in range(KS):
                        sl = slice(k * S + nt * 512, k * S + nt * 512 + 512)
                        tsq = lnp.tile([P, 512], BF, tag="ln_tsq",
                                       name="ln_tsq", bufs=2)
                        nc.vector.tensor_tensor(tsq[:], t16[:, sl],
                                                t16[:, sl], op=ALU.mult)
                        nc.tensor.matmul(
                            pmu[:], self.ones_mean[:, 0:1], t16[:, sl],
                            start=(k == 0), stop=(k == KS - 1))
                        nc.tensor.matmul(
                            psq[:], self.ones_mean[:, 0:1], tsq[:],
                            start=(k == 0), stop=(k == KS - 1))
                    osl = slice(nt * 512, nt * 512 + 512)
                    nc.vector.tensor_copy(mu[:, osl], pmu[:])
                    mu2 = lnp.tile([1, 512], F32, tag="ln_mu2", name="ln_mu2")
                    nc.vector.tensor_tensor(mu2[:], mu[:, osl], mu[:, osl],
                                            op=ALU.mult)
                    nc.vector.tensor_tensor(var[:, osl], psq[:], mu2[:],
                                            op=ALU.subtract)
            # rstd = exp(-0.5*ln(var+eps)); rows, broadcasts and the
            # normalize interior all split by token half so consumers
            # (cross projections, FFN, output DMA) start at half-time
            idt = F32 if hi_fi else BF
            lnv = lnp.tile([1, S], F32, tag="ln_row", name="ln_lnv",
                           bufs=2)
            rstd = lnp.tile([1, S], F32, tag="ln_row", name="ln_rstd",
                            bufs=2)
            if not hi_fi:
                murow = lnp.tile([1, S], BF, tag="ln_mu16")
                rsrow = lnp.tile([1, S], BF, tag="ln_rstd16")
            mub = lnp.tile([P, S], idt, tag="ln_mub")
            rstdb = lnp.tile([P, S], idt, tag="ln_rstdb")
            nsrc = t32 if hi_fi else t16
            for nt in range(2):
                osl = slice(nt * 512, nt * 512 + 512)
                nc.scalar.activation(lnv[:, osl], var[:, osl], AF.Ln,
                                     bias=self.eps_c[:, 0:1])
                nc.scalar.activation(rstd[:, osl], lnv[:, osl], AF.Exp,
                                     scale=-0.5)
                if hi_fi:
                    mr, rr_ = mu, rstd
                else:
                    nc.vector.tensor_copy(murow[:, osl], mu[:, osl])
                    nc.vector.tensor_copy(rsrow[:, osl], rstd[:, osl])
                    mr, rr_ = murow, rsrow
                nc.gpsimd.partition_broadcast(mub[:, osl], mr[:, osl])
                nc.gpsimd.partition_broadcast(rstdb[:, osl], rr_[:, osl])
                for k in range(KS):
                    sl = slice(k * S + nt * 512, k * S + nt * 512 + 512)
                    w = lnp.tile([P, 512], idt, tag="ln_w", name="ln_w",
                                 bufs=3)
                    nc.vector.tensor_tensor(w[:], nsrc[:, sl], mub[:, osl],
                                            op=ALU.subtract)
                    nc.vector.tensor_tensor(w[:], w[:], rstdb[:, osl],
                                            op=ALU.mult)
                    if out32 is not None:
                        nc.vector.tensor_scalar(
                            out32[:, sl], w[:], gam[:, k: k + 1],
                            bet32[:, k: k + 1], op0=ALU.mult, op1=ALU.add)
                    if out16 is not None:
                        nc.vector.tensor_scalar(
                            out16[:, sl], w[:], gam[:, k: k + 1],
                            bet[:, k: k + 1], op0=ALU.mult, op1=ALU.add)
                    if out8 is not None:
                        eng8 = (nc.vector if out32 is None and
                                out16 is None else nc.gpsimd)
                        eng8.tensor_scalar(
                            out8[:, sl], w[:], gam[:, k: k + 1],
                            bet[:, k: k + 1], op0=ALU.mult, op1=ALU.add)
                if out32 is not None and out_dma is not None:
                    nc.sync.dma_start(
                        out_dma.rearrange("(s p) t -> p s t", p=P)
                        [:, :, nt * 512: nt * 512 + 512],
                        out32[:].rearrange("p (s t) -> p s t", s=KS)
                        [:, :, nt * 512: nt * 512 + 512])


    # ---------- main ----------

    def run(self, xo32, xt16, xo8, xt8, attw, f1w, f1b, f2w, gw, gbd,
            nrm, out_t):
        nc, tc, ctx = self.nc, self.tc, self.ctx

        const = ctx.enter_context(tc.tile_pool(name="const", bufs=1))

        self.ones_mean = const.tile([P, 1], BF)
        nc.vector.memset(self.ones_mean[:], 1.0 / E)
        self.eps_c = const.tile([1, 1], F32)
        nc.vector.memset(self.eps_c[:], EPS)
        self.lnc_c = const.tile([P, 1], F32)
        nc.vector.memset(self.lnc_c[:], LN_C)
        # ---- weight prefetch: set 'a' first, then the stage-1 inputs
        # (unblocking the first projections ASAP), then the rest ----
        wp = ctx.enter_context(tc.tile_pool(name="wp_all", bufs=1))
        act = ctx.enter_context(tc.tile_pool(name="acts", bufs=1))
        oup = ctx.enter_context(tc.tile_pool(name="oup", bufs=1))

        W = {}

        def load_set(tag):
            for m in ("qw", "kw", "vw", "ow"):
                W[tag + m] = self.load_w8(wp, attw[tag + m], E, E, tag + m)
            W[tag + "qb"] = self.load_vec(wp, attw[tag + "qb"], KS,
                                          tag + "qb")

        load_set("a")

        # ---- stage 1: self-attention + LN for both streams ----
        s1 = ExitStack()
        pools = {st: s1.enter_context(tc.tile_pool(name="sb_" + st,
                                                   bufs=1))
                 for st in ("own", "oth")}
        s1x = ExitStack()
        x8p = s1x.enter_context(tc.tile_pool(name="s1x", bufs=1))
        xin = {}
        # fp8 matmul inputs first (startup-critical); the fat fp32/bf16
        # residual streams aren't read until the first out-projection
        # (~60us in) so they queue after the weight sets
        for st, (x32d, x8d) in (("own", (xo32, xo8)),
                                ("oth", (xt16, xt8))):
            x8 = x8p.tile([P, KS * S], F8, tag="x8", name="x8" + st,
                          bufs=2)
            nc.sync.dma_start(
                x8[:].rearrange("p (s t) -> p s t", s=KS),
                x8d.rearrange("(s p) t -> p s t", p=P))
            xin[st] = [pools[st], x8, None]

        load_set("b")
        load_set("c")
        for st, x32d in (("own", xo32), ("oth", xt16)):
            xdt = F32 if st == "own" else BF
            x32 = pools[st].tile([P, KS * S], xdt, tag="x32", name="x32")
            nc.sync.dma_start(
                x32[:].rearrange("p (s t) -> p s t", s=KS),
                x32d.rearrange("(s p) t -> p s t", p=P))
            xin[st][2] = x32
        W["cob"] = self.load_vec(wp, attw["cob"], KS, "cob")
        # slab stride padded to 16B: dual-fp8 Ldweights pair-stride rule
        gw_sb = wp.tile([P, 8 * 16], F8, tag="gw")
        nc.sync.dma_start(
            gw_sb[:].rearrange("p (s o) -> p s o", s=8)[:, :, 0:2],
            gw.rearrange("(s p) o -> p s o", p=P))
        # norm params / gate consts aren't needed until the first LN
        # (~100us in) — load them after the startup-critical DMAs
        self.gbdneg = const.tile([1, 1], F32)
        nc.sync.dma_start(self.gbdneg[:], gbd[:])
        nc.vector.tensor_scalar(self.gbdneg[:], self.gbdneg[:], -1.0, None,
                                op0=ALU.mult)
        gam = {t: self.load_vec(const, nrm[t + "g"], KS, name=t + "g")
               for t in ("nao", "nat", "nb", "nc")}
        bet = {t: self.load_vec(const, nrm[t + "b"], KS, name=t + "b")
               for t in ("nao", "nat", "nb", "nc")}
        bet["nb2"] = self.load_vec(const, nrm["nbb2"], KS, name="nb2")

        ou1 = {}
        for st, wtag in (("own", "a"), ("oth", "b")):
            sbp, x8, x32 = xin[st]
            ou1[st] = self.attention_core(
                st, x8, x8, W[wtag + "qw"], W[wtag + "kw"],
                W[wtag + "vw"], W[wtag + "qb"], oup)
        s1x.close()

        y16 = None
        y8 = {}
        for st, (wtag, ntag) in (("own", ("a", "nao")),
                                 ("oth", ("b", "nat"))):
            sbp, x8, x32 = xin[st]
            t1 = x32  # residual accumulates in place over the input
            # residual is pre-scaled x16 with ob folded in on the host;
            # psum is 256*(o@ow), so t1 = 16*(true t1).  LN is
            # scale-invariant.

            def ev_out(ps, ms, _t1=t1):
                sl = slice(ms * S, ms * S + S)
                nc.vector.scalar_tensor_tensor(
                    _t1[:, sl], ps[:], 1.0 / WS, _t1[:, sl],
                    op0=ALU.mult, op1=ALU.add)

            self.attention_finish(ou1[st], W[wtag + "ow"], ev_out)
            if st == "own":
                y16 = act.tile([P, KS * S], BF, tag="a16",
                               name="yo16", bufs=2)
                y8[st] = act.tile([P, KS * S], F8, tag="a8",
                                  name="yo8", bufs=3)
                self.layer_norm(t1, gam[ntag], bet[ntag], None, y8[st],
                                out16=y16)
            else:
                y8[st] = act.tile([P, KS * S], F8, tag="a8",
                                  name="yt8", bufs=3)
                self.layer_norm(t1, gam[ntag], bet[ntag], None, y8[st],
                                bf_in=True)
        s1.close()

        # ---- stage 2: cross attention ----
        # FFN weights load here: early enough to overlap, after the
        # stage-1 SBUF peak has passed.
        wpf = ctx.enter_context(tc.tile_pool(name="wp_ffn", bufs=1))
        w1 = self.load_w8(wpf, f1w, E, HID, "w1", dty=BF)
        b1 = self.load_vec(wpf, f1b, HKS, "b1")
        w2 = self.load_w8(wpf, f2w, HID, E, "w2")

        cross16 = act.tile([P, KS * S], BF, tag="a16", bufs=2)
        cross8 = act.tile([P, KS * S], F8, tag="a8", bufs=3)
        with ExitStack() as sctx:
            sbp = sctx.enter_context(tc.tile_pool(name="sb_c", bufs=1))
            ob = W["cob"]

            ouc = self.attention_core(
                "cross", y8["own"], y8["oth"], W["cqw"], W["ckw"],
                W["cvw"], W["cqb"], oup)

            def ev_cross(ps, ms, _ob=ob):
                sl = slice(ms * S, ms * S + S)
                nc.vector.tensor_scalar(
                    cross16[:, sl], ps[:], 1.0 / (WS * WS),
                    _ob[:, ms: ms + 1], op0=ALU.mult, op1=ALU.add)
                nc.vector.tensor_copy(cross8[:, sl], cross16[:, sl])

            self.attention_finish(ouc, W["cow"], ev_cross, bufs=4)

        # ---- stage 3: gate + merge + LN_b ----
        with ExitStack() as sctx:
            sbp = sctx.enter_context(tc.tile_pool(name="sb_g", bufs=1))
            g0row = sbp.tile([1, S], BF, tag="g0")
            gwr = gw_sb[:].rearrange("p (s o) -> p s o", s=8)  # o padded 16
            self.poolid += 1
            gp = sctx.enter_context(tc.tile_pool(
                name=f"gp{self.poolid}", bufs=2, space="PSUM"))
            srcs = (y8["own"], cross8)
            for nt in range(2):
                l0 = gp.tile([1, 512], F32, tag="gl", name="gl0")
                l1 = gp.tile([1, 512], F32, tag="gl", name="gl1")
                for half in range(2):  # 0: own slabs 0-3, 1: cross 4-7
                    src = srcs[half]
                    xr = src[:].rearrange("p (s t) -> p s t", s=KS)
                    for kp in range(2):
                        for col, l in ((0, l0), (1, l1)):
                            nc.tensor.matmul(
                                l[:],
                                gwr[:, half * 4 + 2 * kp:
                                    half * 4 + 2 * kp + 2, col: col + 1],
                                xr[:, 2 * kp: 2 * kp + 2,
                                   nt * 512: nt * 512 + 512],
                                start=(half == 0 and kp == 0),
                                stop=(half == 1 and kp == 1),
                                perf_mode=PM.DoubleRow)
                l0s = sbp.tile([1, 512], F32, tag="gl0s", name="gl0s")
                nc.vector.tensor_copy(l0s[:], l0[:])
                d = sbp.tile([1, 512], F32, tag="gd", name="gd")
                nc.vector.tensor_tensor(d[:], l1[:], l0s[:],
                                        op=ALU.subtract)
                # g0 = sigmoid(l0-l1+gbd) = 1/(1+exp(l1-l0-gbd))
                eneg = sbp.tile([1, 512], F32, tag="ge", name="ge")
                nc.scalar.activation(eneg[:], d[:], AF.Exp,
                                     scale=1.0 / WS,
                                     bias=self.gbdneg[:, 0:1])
                den = sbp.tile([1, 512], F32, tag="gden", name="gden")
                nc.vector.tensor_scalar(den[:], eneg[:], 1.0, None,
                                        op0=ALU.add)
                with nc.allow_low_precision(
                        reason="gate weight to bf16 is plenty"):
                    nc.vector.reciprocal(
                        g0row[:, nt * 512: nt * 512 + 512], den[:])
            g0b = sbp.tile([P, S], BF, tag="g0b")
            nc.gpsimd.partition_broadcast(g0b[:], g0row[:])
            t2 = sbp.tile([P, KS * S], BF, tag="t2")
            for k in range(KS):
                sl = slice(k * S, k * S + S)
                w = sbp.tile([P, S], BF, tag="gs", name="gs", bufs=2)
                nc.vector.tensor_tensor(w[:], y16[:, sl],
                                        cross16[:, sl], op=ALU.subtract)
                nc.vector.tensor_tensor(w[:], w[:], g0b[:], op=ALU.mult)
                nc.vector.tensor_tensor(t2[:, sl], w[:], cross16[:, sl],
                                        op=ALU.add)
            z32 = act.tile([P, KS * S], F32, tag="a32", bufs=2)
            z16 = act.tile([P, KS * S], BF, tag="a16", bufs=2)
            # z32 carries beta+f2b (so the FFN residual add needs no
            # separate bias); z16 (the FFN input) uses the true beta.
            self.layer_norm(t2, gam["nb"], bet["nb"], z32, z16,
                            bet32=bet["nb2"], bf_in=True)

        # ---- stage 4: FFN (fp8) + LN_c + output ----
        with ExitStack() as sctx:
            sbp = sctx.enter_context(tc.tile_pool(name="sb_f", bufs=1))
            t3 = z32  # FFN residual accumulates in place over z32
            with ExitStack() as fctx:
                hp = fctx.enter_context(tc.tile_pool(name="hp_f", bufs=1))
                h8 = hp.tile([P, HKS * S], F8, tag="h8")

                def ev_gelu(ps, ms):
                    nc.scalar.activation(
                        h8[:, ms * S: ms * S + S],
                        ps[:], AF.Gelu, bias=b1[:, ms: ms + 1])

                self.proj_bf(w1, z16, E, HID, ev_gelu)

                def ev_f2(ps, ms, half):
                    sl = slice(ms * S + half * 512,
                               ms * S + half * 512 + 512)
                    nc.vector.scalar_tensor_tensor(
                        t3[:, sl], ps[:, half * 512: half * 512 + 512],
                        1.0 / WS, z32[:, sl], op0=ALU.mult, op1=ALU.add)

                self.proj_f8(w2, h8, HID, E, ev_f2, halves=True)

            out32 = sbp.tile([P, KS * S], F32, tag="out32")
            self.layer_norm(t3, gam["nc"], bet["nc"], out32, None,
                            out_dma=out_t, hi_fi=True)


_NC_CACHE = {}


def _get_nc(scale):
    key = round(float(scale), 12)
    if key not in _NC_CACHE:
        _NC_CACHE[key] = _build_nc(scale)
    return _NC_CACHE[key]


def _prep_in_maps(inputs):
    """Slice/transform the full inputs into 8 per-core input dicts."""
    f32 = np.float32
    body = np.asarray(inputs["body_feats"], f32)
    limb = np.asarray(inputs["limb_feats"], f32)
    qw = np.asarray(inputs["attn_qw"], f32)
    qb = np.asarray(inputs["attn_qb"], f32)
    kw = np.asarray(inputs["attn_kw"], f32)
    vw = np.asarray(inputs["attn_vw"], f32)
    vb = np.asarray(inputs["attn_vb"], f32)
    ow = np.asarray(inputs["attn_ow"], f32)
    ob = np.asarray(inputs["attn_ob"], f32)
    f1w = np.asarray(inputs["ffn_w1"], f32)
    f1b = np.asarray(inputs["ffn_b1"], f32)
    f2w = np.asarray(inputs["ffn_w2"], f32)
    f2b = np.asarray(inputs["ffn_b2"], f32)
    ns = np.asarray(inputs["norm_scale"], f32)
    nb = np.asarray(inputs["norm_bias"], f32)
    gw = np.asarray(inputs["gate_w"], f32)
    gb = np.asarray(inputs["gate_b"], f32)

    feats = [body, limb]
    ob_eff = [ob[i] + vb[i] @ ow[i] for i in range(4)]
    gbd = np.array([[gb[0] - gb[1]]], f32)
    ln_a = [0, 3]
    ln_c = [2, 5]

    in_maps = []
    for c in range(8):
        b, s = c // 2, c % 2
        o = s          # own stream / self-attn set
        t = 1 - s      # other stream
        cr = 2 + s     # cross-attn set
        xoT = np.ascontiguousarray(feats[o][b].T)
        xtT = np.ascontiguousarray(feats[t][b].T)
        m = {
            # residual streams pre-scaled x16 with the effective out-proj
            # bias folded in (the stage-1 evict adds them to 256x psums
            # scaled by 1/16; LN is scale-invariant)
            "xo32": WS * (xoT + ob_eff[o][:, None]),
            "xt16": (WS * (xtT + ob_eff[t][:, None])).astype(BF16),
            "xo8": xoT.astype(F8NP),
            "xt8": xtT.astype(F8NP),
            "f1w": f1w[s].astype(BF16), "f1b": f1b[s],
            "f2w": (WS * f2w[s]).astype(F8NP), "f2b": f2b[s],
            "gw": (WS * gw).astype(F8NP), "gbd": gbd,
            "naog": ns[ln_a[o]], "naob": nb[ln_a[o]],
            "natg": ns[ln_a[t]], "natb": nb[ln_a[t]],
            "nbg": ns[1], "nbb": nb[1],
            "nbb2": nb[1] + f2b[s],
            "ncg": ns[ln_c[s]], "ncb": nb[ln_c[s]],
            "cob": ob_eff[cr],
        }
        for tag, i in (("a", o), ("b", t), ("c", cr)):
            m[tag + "qw"] = (WS * qw[i]).astype(F8NP)
            m[tag + "kw"] = (WS * kw[i]).astype(F8NP)
            m[tag + "vw"] = (WS * vw[i]).astype(F8NP)
            m[tag + "ow"] = (WS * ow[i]).astype(F8NP)
            m[tag + "qb"] = WS * qb[i]
        in_maps.append(m)
    return in_maps


def kernel(**inputs):
    temp = float(np.asarray(inputs["temperature"]))
    scale = (D ** -0.5) / temp
    nc = _get_nc(scale)
    in_maps = _prep_in_maps(inputs)
    res = run_bass_kernel_spmd(nc, in_maps, core_ids=list(range(8)))
    body = np.empty((B, S, E), np.float32)
    limb = np.empty((B, S, E), np.float32)
    for c in range(8):
        b, s = c // 2, c % 2
        o = res.results[c]["outT"].T
        (body if s == 0 else limb)[b] = o
    return body, limb


# revision 35
# speedup vs baseline: 1.0019x; 1.0019x over previous
"""Trainium2 Bass kernel for nn_DualAttentionLayer (dense dual-stream
transformer layer: 2x self-attention -> cross-attention -> gated merge ->
FFN, with layernorms).

Sharding: 8 cores = 4 batches x 2 streams. Core c handles batch c//2,
stream c%2 (0=body, 1=limb). Each core redundantly computes BOTH streams'
self-attention+LN stage (so no inter-core communication is needed), then
its own stream's cross-attention, gate, FFN and final norms.

v3: fp8 everywhere on the PE + engine rebalance.
 - All projections (QKV, out-proj, FFN w1/w2, gate) run fp8e4m3 with
   DoubleRow perf mode.  Weights pre-scaled x16 on the host; scale
   factors fold into eviction scales / softmax exp scale / LN betas.
 - Self-attn out-proj biases fold into the host-side residual streams.
 - FFN w2 bias folds into LN_b's fp32-path beta.
 - Scores stay bf16 (K=64 per head).  exp() writes fp8e5 u; softmax
   rowsums via a ones-column in V; per-head batched reciprocal (bf16)
   + gpsimd partition broadcast; normalize applies out-of-place into a
   fp8 ou tile that feeds the fp8 out-proj.
 - LayerNorm stats via bf16 ones-matmul; normalize interior in bf16
   (from the bf16 stats copy) even when a fp32 output is requested;
   squares for variance run on Pool.
 - PSUM eviction tiles widened to [128, 1024] (fewer, larger DVE/ACT
   evictions).
"""

import math
import numpy as np
from contextlib import ExitStack

import concourse.bacc as bacc
import concourse.bass as bass
import concourse.mybir as mybir
import concourse.tile as tile
from concourse.bass_utils import run_bass_kernel_spmd

dt = mybir.dt
AF = mybir.ActivationFunctionType
ALU = mybir.AluOpType
PM = mybir.MatmulPerfMode
BF16 = dt.np(dt.bfloat16)
F8NP = dt.np(dt.float8e4)

B, S, E, NH, D = 4, 1024, 512, 8, 64
HID = 4 * E
P = 128
KS = E // P          # 4 feature slabs of 128
MT = S // P          # 8 token m-tiles of 128
HKS = HID // P       # 16 hidden slabs
EPS = 1e-5
WS = 16.0            # host-side fp8 weight scale
C_EXP = 16.0         # softmax exp output scale (cancels in normalization)
LN_C = math.log(C_EXP)
VB = NH * 65 + 8     # v block stride per k-tile, padded to 528:
                     # dual-fp8 Ldweights needs pair stride % 16 == 0

F32 = dt.float32
BF = dt.bfloat16
F8 = dt.float8e4
F8U = dt.float8e5


def _build_nc(scale: float):
    nc = bacc.Bacc("TRN2", target_bir_lowering=False, debug=False,
                   num_devices=8)

    def din(name, shape, dty=F32):
        return nc.dram_tensor(name, shape, dty, kind="ExternalInput").ap()

    # activations (pre-transposed on host, feature-major [E, S])
    xo32 = din("xo32", [E, S])          # own residual = 16*(x+ob_eff), fp32
    xt16 = din("xt16", [E, S], BF)      # oth residual = 16*(x+ob_eff), bf16
    xo8 = din("xo8", [E, S], F8)        # own, fp8 (matmul rhs)
    xt8 = din("xt8", [E, S], F8)

    # attention weight sets: a = self-own, b = self-other, c = cross
    # all fp8 (x16)
    attw = {}
    for tag in ("a", "b", "c"):
        for m in ("qw", "kw", "vw", "ow"):
            attw[tag + m] = din(tag + m, [E, E], F8)
        attw[tag + "qb"] = din(tag + "qb", [E])     # 16*qb
    attw["cob"] = din("cob", [E])       # cross ob + vb@ow (unscaled)

    f1w = din("f1w", [E, HID], BF)      # bf16 (fp8 here costs too much err)
    f1b = din("f1b", [HID])
    f2w = din("f2w", [HID, E], F8)      # 16*w2
    f2b = din("f2b", [E])               # unused on device (folded in nbg2)
    gw = din("gw", [2 * E, 2], F8)
    gbd = din("gbd", [1, 1])            # gate_b[0] - gate_b[1]

    # norm params: a_own, a_oth (post-self-attn), b (post-gate), c (post-ffn)
    # "nb" has two betas: nbb2 = beta + f2b (for the fp32 residual copy)
    nrm = {}
    for tag in ("nao", "nat", "nb", "nc"):
        nrm[tag + "g"] = din(tag + "g", [E])
        nrm[tag + "b"] = din(tag + "b", [E])
    nrm["nbb2"] = din("nbb2", [E])

    out_t = nc.dram_tensor("outT", [E, S], F32, kind="ExternalOutput").ap()

    with TileKernel(nc, scale) as tk:
        tk.run(xo32, xt16, xo8, xt8, attw, f1w, f1b, f2w, gw, gbd,
               nrm, out_t)

    nc.finalize()
    return nc


class TileKernel:
    def __init__(self, nc, scale):
        self.nc = nc
        self.scale = float(scale)
        self.ctx = ExitStack()
        self.poolid = 0

    def __enter__(self):
        self.tc = self.ctx.enter_context(tile.TileContext(self.nc))
        return self

    def __exit__(self, *a):
        return self.ctx.__exit__(*a)

    # ---------- helpers ----------

    def load_vec(self, pool, dram_ap, n, name=None):
        """Load a [n*128] fp32 vector as [128, n] (slab per column)."""
        t = pool.tile([P, n], F32, tag=name)
        self.nc.sync.dma_start(
            t[:], dram_ap.rearrange("(s p) -> p s", p=P))
        return t

    def load_w8(self, pool, dram_ap, in_dim, out_dim, name=None, dty=F8):
        """Load weight [in,out] as [128, (in/128)*out] slab-major."""
        ks = in_dim // P
        t = pool.tile([P, ks * out_dim], dty, tag=name)
        self.nc.sync.dma_start(
            t[:].rearrange("p (s o) -> p s o", s=ks),
            dram_ap.rearrange("(s p) o -> p s o", p=P))
        return t

    def proj_bf(self, wsb, rhs16, in_dim, out_dim, evict, bufs=4):
        """bf16 Form-B projection: out^T = W^T @ x^T."""
        nc = self.nc
        self.poolid += 1
        with self.tc.tile_pool(name=f"pb{self.poolid}", bufs=bufs,
                               space="PSUM") as pp:
            nks = in_dim // P
            wr = wsb[:].rearrange("p (s o) -> p s o", s=nks)
            xr = rhs16[:].rearrange("p (s t) -> p s t", s=nks)
            for ms in range(out_dim // P):
                ps = pp.tile([P, S], F32, tag="proj", name="proj")
                for win in range(2):
                    o = ps[:, win * 512: win * 512 + 512]
                    for k in range(nks):
                        nc.tensor.matmul(
                            o, wr[:, k, ms * P: ms * P + P],
                            xr[:, k, win * 512: win * 512 + 512],
                            start=(k == 0), stop=(k == nks - 1))
                evict(ps, ms)

    def proj_f8(self, wsb, rhs8, in_dim, out_dim, evict, bufs=4,
                halves=False):
        """out^T[out,tok] = (W^T @ x^T) with fp8 DoubleRow matmuls.
        evict(ps, ms) consumes a [128, S] fp32 PSUM tile (full token
        range); with halves=True, evict(ps, ms, half) gets 512-token
        halves as their accumulation chains finish."""
        nc = self.nc
        self.poolid += 1
        with self.tc.tile_pool(name=f"pp{self.poolid}", bufs=bufs,
                               space="PSUM") as pp:
            self._proj_f8(pp, wsb, rhs8, in_dim, out_dim, evict,
                          halves=halves)

    def _proj_f8(self, pp, wsb, rhs8, in_dim, out_dim, evict,
                 halves=False):
        nc = self.nc
        nks = in_dim // P
        npr = nks // 2
        wr = wsb[:].rearrange("p (s o) -> p s o", s=nks)
        xr = rhs8[:].rearrange("p (s t) -> p s t", s=nks)
        for ms in range(out_dim // P):
            ps = pp.tile([P, S], F32, tag="proj", name="proj")
            for win in range(S // 256):
                o = ps[:, win * 256: win * 256 + 256]
                toff = win * 256
                for kp in range(npr):
                    nc.tensor.matmul(
                        o,
                        wr[:, 2 * kp: 2 * kp + 2, ms * P: ms * P + P],
                        xr[:, 2 * kp: 2 * kp + 2, toff: toff + 256],
                        start=(kp == 0), stop=(kp == npr - 1),
                        perf_mode=PM.DoubleRow)
                if halves and win % 2 == 1:
                    evict(ps, ms, win // 2)
            if not halves:
                evict(ps, ms)

    # ---------- attention ----------

    def attention_core(self, name, q8, kv8, wq, wk, wv, qb, oup,
                       kv_first=False):
        """MHA core: fp8 projections, bf16 scores, fp8 exp, DoubleRow AV,
        streamed softmax normalization.  Returns the normalized per-head
        output ou8 (fp8, tile in caller pool `oup`, = 16x true o).
        The caller runs the fp8 out-projection separately
        (attention_finish) so the next attention's core can overlap this
        one's tail."""
        nc, tc = self.nc, self.tc
        ou = oup.tile([P, KS * S], F8, tag="ou", name="ou_" + name,
                      bufs=2)
        with ExitStack() as actx:
            ap = actx.enter_context(
                tc.tile_pool(name="attc_" + name, bufs=1))
            up = actx.enter_context(
                tc.tile_pool(name="attu_" + name, bufs=3))

            qt = ap.tile([P, KS * S], BF, tag="qT")
            kt = ap.tile([P, KS * S], BF, tag="kT")
            vt = ap.tile([P, MT * VB], F8, tag="vT")

            def ev_q(ps, ms):
                nc.vector.tensor_scalar(
                    qt[:, ms * S: ms * S + S],
                    ps[:], qb[:, ms: ms + 1], None, op0=ALU.add)

            def ev_k(ps, ms):
                nc.vector.tensor_copy(
                    kt[:, ms * S: ms * S + S], ps[:])

            self.poolid += 1
            with tc.tile_pool(name=f"attn_pp{self.poolid}", bufs=3,
                              space="PSUM") as pp, \
                 tc.tile_pool(name=f"attn_pv{self.poolid}", bufs=2,
                              space="PSUM") as pv:
                if kv_first:
                    # cross-attn: K/V inputs (the other stream's LN) are
                    # ready before Q's — emit them first
                    self._proj_f8(pp, wk, kv8, E, E, ev_k)
                    self._proj_f8(pp, wq, q8, E, E, ev_q)
                else:
                    self._proj_f8(pp, wq, q8, E, E, ev_q)
                    self._proj_f8(pp, wk, kv8, E, E, ev_k)

                # V: Form A (x^T as lhsT) -> token-major v [tok, feat],
                # strided into per-head 65-wide blocks, col 64 = 1.
                v4 = vt[:].rearrange("p (m c) -> p m c", m=MT)
                nc.gpsimd.memset(
                    v4[:, :, 0:NH * 65]
                    .rearrange("p m (h c) -> p m h c", h=NH)
                    [:, :, :, 64:65], 1.0)
                xr = kv8[:].rearrange("p (s t) -> p s t", s=KS)
                wvr = wv[:].rearrange("p (s o) -> p s o", s=KS)
                for mt in range(MT):
                    ps = pv.tile([P, 512], F32, tag="vproj", name="vproj")
                    for fw in range(2):
                        o = ps[:, fw * 256: fw * 256 + 256]
                        for kp in range(2):
                            nc.tensor.matmul(
                                o,
                                xr[:, 2 * kp: 2 * kp + 2, mt * P: mt * P + P],
                                wvr[:, 2 * kp: 2 * kp + 2,
                                    fw * 256: fw * 256 + 256],
                                start=(kp == 0), stop=(kp == 1),
                                perf_mode=PM.DoubleRow)
                    nc.scalar.copy(
                        v4[:, mt, 0:NH * 65]
                        .rearrange("p (h c) -> p h c", h=NH)[:, :, 0:64],
                        ps[:].rearrange("p (h d) -> p h d", h=NH))

            self.poolid += 1
            scav = ExitStack()
            sp = scav.enter_context(
                tc.tile_pool(name=f"attn_sc{self.poolid}", bufs=2,
                             space="PSUM"))
            avp = scav.enter_context(
                tc.tile_pool(name=f"attn_av{self.poolid}", bufs=4,
                             space="PSUM"))

            v4 = vt[:].rearrange("p (m c) -> p m c", m=MT)
            sexp = self.scale / (WS * WS)
            # head pairs outer: each pair's softmax rowsums are
            # reciprocal'd + broadcast + applied as soon as the pair is
            # done, overlapping the remaining pairs' scores/exp/AV.
            NT = 2
            for j in range(NH // 2):
                hs = (2 * j, 2 * j + 1)
                # per-pair staging across both qn halves: head h%2==0 in
                # partitions 0-63, h%2==1 in 64-127 (TensorTensor needs
                # matching operand base partitions)
                tmp = up.tile([P, S], BF, tag="avtmp", name="avtmp",
                              bufs=2)
                st = {h: up.tile([1, S], F32, tag="rstage",
                                 name=f"rstage{h % 2}", bufs=4)
                      for h in hs}
                for qn in range(NT):
                    av = {(h, w): avp.tile([65, 256], F32, tag="av",
                                           name=f"av{h}_{w}")
                          for h in hs for w in range(2)}
                    for wave in range(MT // 2):
                        sc = {h: sp.tile([P, 1024], F32, tag="sc",
                                         name=f"sc{h}") for h in hs}
                        for i in range(2):
                            mt = wave * 2 + i
                            for h in hs:
                                bp = (h % 2) * 64
                                sl = h // 2
                                nc.tensor.matmul(
                                    sc[h][:, i * 512: i * 512 + 512],
                                    kt[bp: bp + 64,
                                       sl * S + mt * P: sl * S + mt * P + P],
                                    qt[bp: bp + 64,
                                       sl * S + qn * 512: sl * S + qn * 512 + 512],
                                    start=True, stop=True)
                        ut = {h: up.tile([P, 1024], F8, tag="u",
                                         name=f"u{h}") for h in hs}
                        for h in hs:
                            nc.scalar.activation(
                                ut[h][:], sc[h][:], AF.Exp, scale=sexp,
                                bias=self.lnc_c[:, 0:1])
                        for h in hs:
                            utr = ut[h][:].rearrange("p (i t) -> p i t", i=2)
                            for win in range(2):
                                nc.tensor.matmul(
                                    av[h, win][:],
                                    v4[:, 2 * wave: 2 * wave + 2,
                                       h * 65: h * 65 + 65],
                                    utr[:, :, win * 256: win * 256 + 256],
                                    start=(wave == 0), stop=(wave == 3),
                                    perf_mode=PM.DoubleRow)
                    for h in hs:
                        bp = (h % 2) * 64
                        # unnormalized o^T and rowsum -> bf16 staging
                        for win in range(2):
                            qo = qn * 512 + win * 256
                            nc.vector.tensor_copy(
                                tmp[bp: bp + 64, qo: qo + 256],
                                av[h, win][0:64, :])
                            nc.vector.tensor_copy(
                                st[h][:, qo: qo + 256],
                                av[h, win][64:65, :])
                for h in hs:
                    bp = (h % 2) * 64
                    sl = h // 2
                    rr = up.tile([1, S], BF, tag="rrec", name="rrec")
                    with nc.allow_low_precision(
                            reason="1/rowsum to bf16 is plenty"):
                        nc.vector.reciprocal(rr[:], st[h][:])
                    # full-128 broadcast: HW ucode mishandles
                    # non-zero destination base partitions
                    rbt = ap.tile([P, S], BF, tag="rbh",
                                  name=f"rbh{h % 2}", bufs=4)
                    nc.gpsimd.partition_broadcast(rbt[:], rr[:])
                    nc.vector.tensor_tensor(
                        ou[bp: bp + 64, sl * S: sl * S + S],
                        tmp[bp: bp + 64, :], rbt[bp: bp + 64, :],
                        op=ALU.mult)
            scav.close()
        return ou

    def attention_finish(self, ou8, wo, evict_out, bufs=2):
        """fp8 out projection of a finished attention core."""
        self.proj_f8(wo, ou8, E, E, evict_out, bufs=bufs)

    # ---------- layernorm ----------

    def layer_norm(self, t32, gam, bet, out32, out8, out16=None,
                   out_dma=None, bf_in=False, bet32=None, hi_fi=False):
        """LN over features (partition axis) of t32 [128, KS*S].
        Stats come from a bf16 copy (ones-matmul over partitions); the
        normalize interior runs in bf16 (or fp32 when hi_fi, for the
        final output norm); the fp32 output (if requested) applies
        gamma/bet32 from the interior.  Optional fp8 side output;
        out_dma streams the fp32 output to DRAM per slab.
        bf_in: t32 is already bf16 (skip the cast)."""
        nc = self.nc
        if bet32 is None:
            bet32 = bet
        self.poolid += 1
        with self.tc.tile_pool(name=f"lnsb{self.poolid}", bufs=1) as lnp:
            if bf_in:
                t16 = t32
            else:
                t16 = lnp.tile([P, KS * S], BF, tag="ln_t16")
                for nt in range(2):
                    for k in range(KS):
                        sl = slice(k * S + nt * 512, k * S + nt * 512 + 512)
                        nc.vector.tensor_copy(t16[:, sl], t32[:, sl])
            mu = lnp.tile([1, S], F32, tag="ln_mu", name="ln_mu")
            var = lnp.tile([1, S], F32, tag="ln_row", name="ln_var",
                           bufs=2)
            self.poolid += 1
            with self.tc.tile_pool(name=f"lnp{self.poolid}", bufs=2,
                                   space="PSUM") as sp1:
                for nt in range(2):
                    pmu = sp1.tile([1, 512], F32, tag="ln_stat", name="pmu")
                    psq = sp1.tile([1, 512], F32, tag="ln_stat", name="psq")
                    for k in range(KS):
                        sl = slice(k * S + nt * 512, k * S + nt * 512 + 512)
                        tsq = lnp.tile([P, 512], BF, tag="ln_tsq",
                                       name="ln_tsq", bufs=2)
                        nc.vector.tensor_tensor(tsq[:], t16[:, sl],
                                                t16[:, sl], op=ALU.mult)
                        nc.tensor.matmul(
                            pmu[:], self.ones_mean[:, 0:1], t16[:, sl],
                            start=(k == 0), stop=(k == KS - 1))
                        nc.tensor.matmul(
                            psq[:], self.ones_mean[:, 0:1], tsq[:],
                            start=(k == 0), stop=(k == KS - 1))
                    osl = slice(nt * 512, nt * 512 + 512)
                    nc.vector.tensor_copy(mu[:, osl], pmu[:])
                    mu2 = lnp.tile([1, 512], F32, tag="ln_mu2", name="ln_mu2")
                    nc.vector.tensor_tensor(mu2[:], mu[:, osl], mu[:, osl],
                                            op=ALU.mult)
                    nc.vector.tensor_tensor(var[:, osl], psq[:], mu2[:],
                                            op=ALU.subtract)
            # rstd = exp(-0.5*ln(var+eps)); rows, broadcasts and the
            # normalize interior all split by token half so consumers
            # (cross projections, FFN, output DMA) start at half-time
            idt = F32 if hi_fi else BF
            lnv = lnp.tile([1, S], F32, tag="ln_row", name="ln_lnv",
                           bufs=2)
            rstd = lnp.tile([1, S], F32, tag="ln_row", name="ln_rstd",
                            bufs=2)
            if not hi_fi:
                murow = lnp.tile([1, S], BF, tag="ln_mu16")
                rsrow = lnp.tile([1, S], BF, tag="ln_rstd16")
            mub = lnp.tile([P, S], idt, tag="ln_mub")
            rstdb = lnp.tile([P, S], idt, tag="ln_rstdb")
            nsrc = t32 if hi_fi else t16
            for nt in range(2):
                osl = slice(nt * 512, nt * 512 + 512)
                nc.scalar.activation(lnv[:, osl], var[:, osl], AF.Ln,
                                     bias=self.eps_c[:, 0:1])
                nc.scalar.activation(rstd[:, osl], lnv[:, osl], AF.Exp,
                                     scale=-0.5)
                if hi_fi:
                    mr, rr_ = mu, rstd
                else:
                    nc.vector.tensor_copy(murow[:, osl], mu[:, osl])
                    nc.vector.tensor_copy(rsrow[:, osl], rstd[:, osl])
                    mr, rr_ = murow, rsrow
                nc.gpsimd.partition_broadcast(mub[:, osl], mr[:, osl])
                nc.gpsimd.partition_broadcast(rstdb[:, osl], rr_[:, osl])
                for k in range(KS):
                    sl = slice(k * S + nt * 512, k * S + nt * 512 + 512)
                    w = lnp.tile([P, 512], idt, tag="ln_w", name="ln_w",
                                 bufs=3)
                    nc.vector.tensor_tensor(w[:], nsrc[:, sl], mub[:, osl],
                                            op=ALU.subtract)
                    nc.vector.tensor_tensor(w[:], w[:], rstdb[:, osl],
                                            op=ALU.mult)
                    if out32 is not None:
                        nc.vector.tensor_scalar(
                            out32[:, sl], w[:], gam[:, k: k + 1],
                            bet32[:, k: k + 1], op0=ALU.mult, op1=ALU.add)
                    if out16 is not None:
                        nc.vector.tensor_scalar(
                            out16[:, sl], w[:], gam[:, k: k + 1],
                            bet[:, k: k + 1], op0=ALU.mult, op1=ALU.add)
                    if out8 is not None:
                        nc.gpsimd.tensor_scalar(
                            out8[:, sl], w[:], gam[:, k: k + 1],
                            bet[:, k: k + 1], op0=ALU.mult, op1=ALU.add)
                    if out32 is not None and out_dma is not None:
                        nc.sync.dma_start(
                            out_dma.rearrange("(s p) t -> p s t", p=P)
                            [:, k, nt * 512: nt * 512 + 512],
                            out32[:, sl])

    # ---------- main ----------

    def run(self, xo32, xt16, xo8, xt8, attw, f1w, f1b, f2w, gw, gbd,
            nrm, out_t):
        nc, tc, ctx = self.nc, self.tc, self.ctx

        const = ctx.enter_context(tc.tile_pool(name="const", bufs=1))

        self.ones_mean = const.tile([P, 1], BF)
        nc.vector.memset(self.ones_mean[:], 1.0 / E)
        self.eps_c = const.tile([1, 1], F32)
        nc.vector.memset(self.eps_c[:], EPS)
        self.lnc_c = const.tile([P, 1], F32)
        nc.vector.memset(self.lnc_c[:], LN_C)
        # ---- weight prefetch: set 'a' first, then the stage-1 inputs
        # (unblocking the first projections ASAP), then the rest ----
        wp = ctx.enter_context(tc.tile_pool(name="wp_all", bufs=1))
        act = ctx.enter_context(tc.tile_pool(name="acts", bufs=1))
        oup = ctx.enter_context(tc.tile_pool(name="oup", bufs=1))

        W = {}

        def load_set(tag):
            for m in ("qw", "kw", "vw", "ow"):
                W[tag + m] = self.load_w8(wp, attw[tag + m], E, E, tag + m)
            W[tag + "qb"] = self.load_vec(wp, attw[tag + "qb"], KS,
                                          tag + "qb")

        load_set("a")

        # ---- stage 1: self-attention + LN for both streams ----
        s1 = ExitStack()
        pools = {st: s1.enter_context(tc.tile_pool(name="sb_" + st,
                                                   bufs=1))
                 for st in ("own", "oth")}
        s1x = ExitStack()
        x8p = s1x.enter_context(tc.tile_pool(name="s1x", bufs=1))
        xin = {}
        # fp8 matmul inputs first (startup-critical); the fat fp32/bf16
        # residual streams aren't read until the first out-projection
        # (~60us in) so they queue after the weight sets
        for st, (x32d, x8d) in (("own", (xo32, xo8)),
                                ("oth", (xt16, xt8))):
            x8 = x8p.tile([P, KS * S], F8, tag="x8", name="x8" + st,
                          bufs=2)
            nc.sync.dma_start(
                x8[:].rearrange("p (s t) -> p s t", s=KS),
                x8d.rearrange("(s p) t -> p s t", p=P))
            xin[st] = [pools[st], x8, None]

        load_set("b")
        load_set("c")
        for st, x32d in (("own", xo32), ("oth", xt16)):
            xdt = F32 if st == "own" else BF
            x32 = pools[st].tile([P, KS * S], xdt, tag="x32", name="x32")
            nc.sync.dma_start(
                x32[:].rearrange("p (s t) -> p s t", s=KS),
                x32d.rearrange("(s p) t -> p s t", p=P))
            xin[st][2] = x32
        W["cob"] = self.load_vec(wp, attw["cob"], KS, "cob")
        # slab stride padded to 16B: dual-fp8 Ldweights pair-stride rule
        gw_sb = wp.tile([P, 8 * 16], F8, tag="gw")
        nc.sync.dma_start(
            gw_sb[:].rearrange("p (s o) -> p s o", s=8)[:, :, 0:2],
            gw.rearrange("(s p) o -> p s o", p=P))
        # norm params / gate consts aren't needed until the first LN
        # (~100us in) — load them after the startup-critical DMAs
        self.gbdneg = const.tile([1, 1], F32)
        nc.sync.dma_start(self.gbdneg[:], gbd[:])
        nc.vector.tensor_scalar(self.gbdneg[:], self.gbdneg[:], -1.0, None,
                                op0=ALU.mult)
        gam = {t: self.load_vec(const, nrm[t + "g"], KS, name=t + "g")
               for t in ("nao", "nat", "nb", "nc")}
        bet = {t: self.load_vec(const, nrm[t + "b"], KS, name=t + "b")
               for t in ("nao", "nat", "nb", "nc")}
        bet["nb2"] = self.load_vec(const, nrm["nbb2"], KS, name="nb2")

        ou1 = {}
        for st, wtag in (("own", "a"), ("oth", "b")):
            sbp, x8, x32 = xin[st]
            ou1[st] = self.attention_core(
                st, x8, x8, W[wtag + "qw"], W[wtag + "kw"],
                W[wtag + "vw"], W[wtag + "qb"], oup)
        s1x.close()

        y16 = None
        y8 = {}
        for st, (wtag, ntag) in (("own", ("a", "nao")),
                                 ("oth", ("b", "nat"))):
            sbp, x8, x32 = xin[st]
            t1 = x32  # residual accumulates in place over the input
            # residual is pre-scaled x16 with ob folded in on the host;
            # psum is 256*(o@ow), so t1 = 16*(true t1).  LN is
            # scale-invariant.

            def ev_out(ps, ms, _t1=t1):
                sl = slice(ms * S, ms * S + S)
                nc.vector.scalar_tensor_tensor(
                    _t1[:, sl], ps[:], 1.0 / WS, _t1[:, sl],
                    op0=ALU.mult, op1=ALU.add)

            self.attention_finish(ou1[st], W[wtag + "ow"], ev_out)
            if st == "own":
                y16 = act.tile([P, KS * S], BF, tag="a16",
                               name="yo16", bufs=2)
                y8[st] = act.tile([P, KS * S], F8, tag="a8",
                                  name="yo8", bufs=3)
                self.layer_norm(t1, gam[ntag], bet[ntag], None, y8[st],
                                out16=y16)
            else:
                y8[st] = act.tile([P, KS * S], F8, tag="a8",
                                  name="yt8", bufs=3)
                self.layer_norm(t1, gam[ntag], bet[ntag], None, y8[st],
                                bf_in=True)
        s1.close()

        # ---- stage 2: cross attention ----
        # FFN weights load here: early enough to overlap, after the
        # stage-1 SBUF peak has passed.
        wpf = ctx.enter_context(tc.tile_pool(name="wp_ffn", bufs=1))
        w1 = self.load_w8(wpf, f1w, E, HID, "w1", dty=BF)
        b1 = self.load_vec(wpf, f1b, HKS, "b1")
        w2 = self.load_w8(wpf, f2w, HID, E, "w2")

        cross16 = act.tile([P, KS * S], BF, tag="a16", bufs=2)
        cross8 = act.tile([P, KS * S], F8, tag="a8", bufs=3)
        with ExitStack() as sctx:
            sbp = sctx.enter_context(tc.tile_pool(name="sb_c", bufs=1))
            ob = W["cob"]

            ouc = self.attention_core(
                "cross", y8["own"], y8["oth"], W["cqw"], W["ckw"],
                W["cvw"], W["cqb"], oup)

            def ev_cross(ps, ms, _ob=ob):
                sl = slice(ms * S, ms * S + S)
                nc.vector.tensor_scalar(
                    cross16[:, sl], ps[:], 1.0 / (WS * WS),
                    _ob[:, ms: ms + 1], op0=ALU.mult, op1=ALU.add)
                nc.gpsimd.tensor_copy(cross8[:, sl], cross16[:, sl])

            self.attention_finish(ouc, W["cow"], ev_cross, bufs=4)

        # ---- stage 3: gate + merge + LN_b ----
        with ExitStack() as sctx:
            sbp = sctx.enter_context(tc.tile_pool(name="sb_g", bufs=1))
            g0row = sbp.tile([1, S], BF, tag="g0")
            gwr = gw_sb[:].rearrange("p (s o) -> p s o", s=8)  # o padded 16
            self.poolid += 1
            gp = sctx.enter_context(tc.tile_pool(
                name=f"gp{self.poolid}", bufs=2, space="PSUM"))
            srcs = (y8["own"], cross8)
            for nt in range(2):
                l0 = gp.tile([1, 512], F32, tag="gl", name="gl0")
                l1 = gp.tile([1, 512], F32, tag="gl", name="gl1")
                for half in range(2):  # 0: own slabs 0-3, 1: cross 4-7
                    src = srcs[half]
                    xr = src[:].rearrange("p (s t) -> p s t", s=KS)
                    for kp in range(2):
                        for col, l in ((0, l0), (1, l1)):
                            nc.tensor.matmul(
                                l[:],
                                gwr[:, half * 4 + 2 * kp:
                                    half * 4 + 2 * kp + 2, col: col + 1],
                                xr[:, 2 * kp: 2 * kp + 2,
                                   nt * 512: nt * 512 + 512],
                                start=(half == 0 and kp == 0),
                                stop=(half == 1 and kp == 1),
                                perf_mode=PM.DoubleRow)
                l0s = sbp.tile([1, 512], F32, tag="gl0s", name="gl0s")
                nc.vector.tensor_copy(l0s[:], l0[:])
                d = sbp.tile([1, 512], F32, tag="gd", name="gd")
                nc.vector.tensor_tensor(d[:], l1[:], l0s[:],
                                        op=ALU.subtract)
                # g0 = sigmoid(l0-l1+gbd) = 1/(1+exp(l1-l0-gbd))
                eneg = sbp.tile([1, 512], F32, tag="ge", name="ge")
                nc.scalar.activation(eneg[:], d[:], AF.Exp,
                                     scale=1.0 / WS,
                                     bias=self.gbdneg[:, 0:1])
                den = sbp.tile([1, 512], F32, tag="gden", name="gden")
                nc.vector.tensor_scalar(den[:], eneg[:], 1.0, None,
                                        op0=ALU.add)
                with nc.allow_low_precision(
                        reason="gate weight to bf16 is plenty"):
                    nc.vector.reciprocal(
                        g0row[:, nt * 512: nt * 512 + 512], den[:])
            g0b = sbp.tile([P, S], BF, tag="g0b")
            nc.gpsimd.partition_broadcast(g0b[:], g0row[:])
            t2 = sbp.tile([P, KS * S], BF, tag="t2")
            for k in range(KS):
                sl = slice(k * S, k * S + S)
                w = sbp.tile([P, S], BF, tag="gs", name="gs", bufs=2)
                nc.vector.tensor_tensor(w[:], y16[:, sl],
                                        cross16[:, sl], op=ALU.subtract)
                nc.vector.tensor_tensor(w[:], w[:], g0b[:], op=ALU.mult)
                nc.vector.tensor_tensor(t2[:, sl], w[:], cross16[:, sl],
                                        op=ALU.add)
            z32 = act.tile([P, KS * S], F32, tag="a32", bufs=2)
            z16 = act.tile([P, KS * S], BF, tag="a16", bufs=2)
            # z32 carries beta+f2b (so the FFN residual add needs no
            # separate bias); z16 (the FFN input) uses the true beta.
            self.layer_norm(t2, gam["nb"], bet["nb"], z32, z16,
                            bet32=bet["nb2"], bf_in=True)

        # ---- stage 4: FFN (fp8) + LN_c + output ----
        with ExitStack() as sctx:
            sbp = sctx.enter_context(tc.tile_pool(name="sb_f", bufs=1))
            t3 = z32  # FFN residual accumulates in place over z32
            with ExitStack() as fctx:
                hp = fctx.enter_context(tc.tile_pool(name="hp_f", bufs=1))
                h8 = hp.tile([P, HKS * S], F8, tag="h8")

                def ev_gelu(ps, ms):
                    nc.scalar.activation(
                        h8[:, ms * S: ms * S + S],
                        ps[:], AF.Gelu, bias=b1[:, ms: ms + 1])

                self.proj_bf(w1, z16, E, HID, ev_gelu)

                def ev_f2(ps, ms, half):
                    sl = slice(ms * S + half * 512,
                               ms * S + half * 512 + 512)
                    nc.vector.scalar_tensor_tensor(
                        t3[:, sl], ps[:, half * 512: half * 512 + 512],
                        1.0 / WS, z32[:, sl], op0=ALU.mult, op1=ALU.add)

                self.proj_f8(w2, h8, HID, E, ev_f2, halves=True)

            out32 = sbp.tile([P, KS * S], F32, tag="out32")
            self.layer_norm(t3, gam["nc"], bet["nc"], out32, None,
                            out_dma=out_t, hi_fi=True)


_NC_CACHE = {}


def _get_nc(scale):
    key = round(float(scale), 12)
    if key not in _NC_CACHE:
        _NC_CACHE[key] = _build_nc(scale)
    return _NC_CACHE[key]


def _prep_in_maps(inputs):
    """Slice/transform the full inputs into 8 per-core input dicts."""
    f32 = np.float32
    body = np.asarray(inputs["body_feats"], f32)
    limb = np.asarray(inputs["limb_feats"], f32)
    qw = np.asarray(inputs["attn_qw"], f32)
    qb = np.asarray(inputs["attn_qb"], f32)
    kw = np.asarray(inputs["attn_kw"], f32)
    vw = np.asarray(inputs["attn_vw"], f32)
    vb = np.asarray(inputs["attn_vb"], f32)
    ow = np.asarray(inputs["attn_ow"], f32)
    ob = np.asarray(inputs["attn_ob"], f32)
    f1w = np.asarray(inputs["ffn_w1"], f32)
    f1b = np.asarray(inputs["ffn_b1"], f32)
    f2w = np.asarray(inputs["ffn_w2"], f32)
    f2b = np.asarray(inputs["ffn_b2"], f32)
    ns = np.asarray(inputs["norm_scale"], f32)
    nb = np.asarray(inputs["norm_bias"], f32)
    gw = np.asarray(inputs["gate_w"], f32)
    gb = np.asarray(inputs["gate_b"], f32)

    feats = [body, limb]
    ob_eff = [ob[i] + vb[i] @ ow[i] for i in range(4)]
    gbd = np.array([[gb[0] - gb[1]]], f32)
    ln_a = [0, 3]
    ln_c = [2, 5]

    in_maps = []
    for c in range(8):
        b, s = c // 2, c % 2
        o = s          # own stream / self-attn set
        t = 1 - s      # other stream
        cr = 2 + s     # cross-attn set
        xoT = np.ascontiguousarray(feats[o][b].T)
        xtT = np.ascontiguousarray(feats[t][b].T)
        m = {
            # residual streams pre-scaled x16 with the effective out-proj
            # bias folded in (the stage-1 evict adds them to 256x psums
            # scaled by 1/16; LN is scale-invariant)
            "xo32": WS * (xoT + ob_eff[o][:, None]),
            "xt16": (WS * (xtT + ob_eff[t][:, None])).astype(BF16),
            "xo8": xoT.astype(F8NP),
            "xt8": xtT.astype(F8NP),
            "f1w": f1w[s].astype(BF16), "f1b": f1b[s],
            "f2w": (WS * f2w[s]).astype(F8NP), "f2b": f2b[s],
            "gw": (WS * gw).astype(F8NP), "gbd": gbd,
            "naog": ns[ln_a[o]], "naob": nb[ln_a[o]],
            "natg": ns[ln_a[t]], "natb": nb[ln_a[t]],
            "nbg": ns[1], "nbb": nb[1],
            "nbb2": nb[1] + f2b[s],
            "ncg": ns[ln_c[s]], "ncb": nb[ln_c[s]],
            "cob": ob_eff[cr],
        }
        for tag, i in (("a", o), ("b", t), ("c", cr)):
            m[tag + "qw"] = (WS * qw[i]).astype(F8NP)
            m[tag + "kw"] = (WS * kw[i]).astype(F8NP)
            m[tag + "vw"] = (WS * vw[i]).astype(F8NP)
            m[tag + "ow"] = (WS * ow[i]).astype(F8NP)
            m[tag + "qb"] = WS * qb[i]
        in_maps.append(m)
    return in_maps


def kernel(**inputs):
    temp = float(np.asarray(inputs["temperature"]))
    scale = (D ** -0.5) / temp
    nc = _get_nc(scale)
    in_maps = _prep_in_maps(inputs)
    res = run_bass_kernel_spmd(nc, in_maps, core_ids=list(range(8)))
    body = np.empty((B, S, E), np.float32)
    limb = np.empty((B, S, E), np.float32)
    for c in range(8):
        b, s = c // 2, c % 2
        o = res.results[c]["outT"].T
        (body if s == 0 else limb)[b] = o
    return body, limb


# revision 39
# speedup vs baseline: 1.0163x; 1.0143x over previous
"""Trainium2 Bass kernel for nn_DualAttentionLayer (dense dual-stream
transformer layer: 2x self-attention -> cross-attention -> gated merge ->
FFN, with layernorms).

Sharding: 8 cores = 4 batches x 2 streams. Core c handles batch c//2,
stream c%2 (0=body, 1=limb). Each core redundantly computes BOTH streams'
self-attention+LN stage (so no inter-core communication is needed), then
its own stream's cross-attention, gate, FFN and final norms.

v3: fp8 everywhere on the PE + engine rebalance.
 - All projections (QKV, out-proj, FFN w1/w2, gate) run fp8e4m3 with
   DoubleRow perf mode.  Weights pre-scaled x16 on the host; scale
   factors fold into eviction scales / softmax exp scale / LN betas.
 - Self-attn out-proj biases fold into the host-side residual streams.
 - FFN w2 bias folds into LN_b's fp32-path beta.
 - Scores stay bf16 (K=64 per head).  exp() writes fp8e5 u; softmax
   rowsums via a ones-column in V; per-head batched reciprocal (bf16)
   + gpsimd partition broadcast; normalize applies out-of-place into a
   fp8 ou tile that feeds the fp8 out-proj.
 - LayerNorm stats via bf16 ones-matmul; normalize interior in bf16
   (from the bf16 stats copy) even when a fp32 output is requested;
   squares for variance run on Pool.
 - PSUM eviction tiles widened to [128, 1024] (fewer, larger DVE/ACT
   evictions).
"""

import math
import numpy as np
from contextlib import ExitStack

import concourse.bacc as bacc
import concourse.bass as bass
import concourse.mybir as mybir
import concourse.tile as tile
from concourse.bass_utils import run_bass_kernel_spmd

dt = mybir.dt
AF = mybir.ActivationFunctionType
ALU = mybir.AluOpType
PM = mybir.MatmulPerfMode
BF16 = dt.np(dt.bfloat16)
F8NP = dt.np(dt.float8e4)

B, S, E, NH, D = 4, 1024, 512, 8, 64
HID = 4 * E
P = 128
KS = E // P          # 4 feature slabs of 128
MT = S // P          # 8 token m-tiles of 128
HKS = HID // P       # 16 hidden slabs
EPS = 1e-5
WS = 16.0            # host-side fp8 weight scale
C_EXP = 16.0         # softmax exp output scale (cancels in normalization)
LN_C = math.log(C_EXP)
VB = NH * 65 + 8     # v block stride per k-tile, padded to 528:
                     # dual-fp8 Ldweights needs pair stride % 16 == 0

F32 = dt.float32
BF = dt.bfloat16
F8 = dt.float8e4
F8U = dt.float8e5


def _build_nc(scale: float):
    nc = bacc.Bacc("TRN2", target_bir_lowering=False, debug=False,
                   num_devices=8)

    def din(name, shape, dty=F32):
        return nc.dram_tensor(name, shape, dty, kind="ExternalInput").ap()

    # activations (pre-transposed on host, feature-major [E, S])
    xo32 = din("xo32", [E, S])          # own residual = 16*(x+ob_eff), fp32
    xt16 = din("xt16", [E, S], BF)      # oth residual = 16*(x+ob_eff), bf16
    xo8 = din("xo8", [E, S], F8)        # own, fp8 (matmul rhs)
    xt8 = din("xt8", [E, S], F8)

    # attention weight sets: a = self-own, b = self-other, c = cross
    # all fp8 (x16)
    attw = {}
    for tag in ("a", "b", "c"):
        for m in ("qw", "kw", "vw", "ow"):
            attw[tag + m] = din(tag + m, [E, E], F8)
        attw[tag + "qb"] = din(tag + "qb", [E])     # 16*qb
    attw["cob"] = din("cob", [E])       # cross ob + vb@ow (unscaled)

    f1w = din("f1w", [E, HID], BF)      # bf16 (fp8 here costs too much err)
    f1b = din("f1b", [HID])
    f2w = din("f2w", [HID, E], F8)      # 16*w2
    f2b = din("f2b", [E])               # unused on device (folded in nbg2)
    gw = din("gw", [2 * E, 2], F8)
    gbd = din("gbd", [1, 1])            # gate_b[0] - gate_b[1]

    # norm params: a_own, a_oth (post-self-attn), b (post-gate), c (post-ffn)
    # "nb" has two betas: nbb2 = beta + f2b (for the fp32 residual copy)
    nrm = {}
    for tag in ("nao", "nat", "nb", "nc"):
        nrm[tag + "g"] = din(tag + "g", [E])
        nrm[tag + "b"] = din(tag + "b", [E])
    nrm["nbb2"] = din("nbb2", [E])

    out_t = nc.dram_tensor("outT", [E, S], F32, kind="ExternalOutput").ap()

    with TileKernel(nc, scale) as tk:
        tk.run(xo32, xt16, xo8, xt8, attw, f1w, f1b, f2w, gw, gbd,
               nrm, out_t)

    nc.finalize()
    return nc


class TileKernel:
    def __init__(self, nc, scale):
        self.nc = nc
        self.scale = float(scale)
        self.ctx = ExitStack()
        self.poolid = 0

    def __enter__(self):
        self.tc = self.ctx.enter_context(tile.TileContext(self.nc))
        return self

    def __exit__(self, *a):
        return self.ctx.__exit__(*a)

    # ---------- helpers ----------

    def load_vec(self, pool, dram_ap, n, name=None):
        """Load a [n*128] fp32 vector as [128, n] (slab per column)."""
        t = pool.tile([P, n], F32, tag=name)
        self.nc.sync.dma_start(
            t[:], dram_ap.rearrange("(s p) -> p s", p=P))
        return t

    def load_w8(self, pool, dram_ap, in_dim, out_dim, name=None, dty=F8):
        """Load weight [in,out] as [128, (in/128)*out] slab-major."""
        ks = in_dim // P
        t = pool.tile([P, ks * out_dim], dty, tag=name)
        self.nc.sync.dma_start(
            t[:].rearrange("p (s o) -> p s o", s=ks),
            dram_ap.rearrange("(s p) o -> p s o", p=P))
        return t

    def proj_bf(self, wsb, rhs16, in_dim, out_dim, evict, bufs=4):
        """bf16 Form-B projection: out^T = W^T @ x^T."""
        nc = self.nc
        self.poolid += 1
        with self.tc.tile_pool(name=f"pb{self.poolid}", bufs=bufs,
                               space="PSUM") as pp:
            nks = in_dim // P
            wr = wsb[:].rearrange("p (s o) -> p s o", s=nks)
            xr = rhs16[:].rearrange("p (s t) -> p s t", s=nks)
            for ms in range(out_dim // P):
                ps = pp.tile([P, S], F32, tag="proj", name="proj")
                for win in range(2):
                    o = ps[:, win * 512: win * 512 + 512]
                    for k in range(nks):
                        nc.tensor.matmul(
                            o, wr[:, k, ms * P: ms * P + P],
                            xr[:, k, win * 512: win * 512 + 512],
                            start=(k == 0), stop=(k == nks - 1))
                evict(ps, ms)

    def proj_f8(self, wsb, rhs8, in_dim, out_dim, evict, bufs=4,
                halves=False):
        """out^T[out,tok] = (W^T @ x^T) with fp8 DoubleRow matmuls.
        evict(ps, ms) consumes a [128, S] fp32 PSUM tile (full token
        range); with halves=True, evict(ps, ms, half) gets 512-token
        halves as their accumulation chains finish."""
        nc = self.nc
        self.poolid += 1
        with self.tc.tile_pool(name=f"pp{self.poolid}", bufs=bufs,
                               space="PSUM") as pp:
            self._proj_f8(pp, wsb, rhs8, in_dim, out_dim, evict,
                          halves=halves)

    def _proj_f8(self, pp, wsb, rhs8, in_dim, out_dim, evict,
                 halves=False):
        nc = self.nc
        nks = in_dim // P
        npr = nks // 2
        wr = wsb[:].rearrange("p (s o) -> p s o", s=nks)
        xr = rhs8[:].rearrange("p (s t) -> p s t", s=nks)
        for ms in range(out_dim // P):
            ps = pp.tile([P, S], F32, tag="proj", name="proj")
            for win in range(S // 256):
                o = ps[:, win * 256: win * 256 + 256]
                toff = win * 256
                for kp in range(npr):
                    nc.tensor.matmul(
                        o,
                        wr[:, 2 * kp: 2 * kp + 2, ms * P: ms * P + P],
                        xr[:, 2 * kp: 2 * kp + 2, toff: toff + 256],
                        start=(kp == 0), stop=(kp == npr - 1),
                        perf_mode=PM.DoubleRow)
                if halves and win % 2 == 1:
                    evict(ps, ms, win // 2)
            if not halves:
                evict(ps, ms)

    # ---------- attention ----------

    def attention_core(self, name, q8, kv8, wq, wk, wv, qb, oup,
                       kv_first=False):
        """MHA core: fp8 projections, bf16 scores, fp8 exp, DoubleRow AV,
        streamed softmax normalization.  Returns the normalized per-head
        output ou8 (fp8, tile in caller pool `oup`, = 16x true o).
        The caller runs the fp8 out-projection separately
        (attention_finish) so the next attention's core can overlap this
        one's tail."""
        nc, tc = self.nc, self.tc
        ou = oup.tile([P, KS * S], F8, tag="ou", name="ou_" + name,
                      bufs=2)
        with ExitStack() as actx:
            ap = actx.enter_context(
                tc.tile_pool(name="attc_" + name, bufs=1))
            up = actx.enter_context(
                tc.tile_pool(name="attu_" + name, bufs=3))

            qt = ap.tile([P, KS * S], BF, tag="qT")
            kt = ap.tile([P, KS * S], BF, tag="kT")
            vt = ap.tile([P, MT * VB], F8, tag="vT")

            def ev_q(ps, ms):
                nc.vector.tensor_scalar(
                    qt[:, ms * S: ms * S + S],
                    ps[:], qb[:, ms: ms + 1], None, op0=ALU.add)

            def ev_k(ps, ms):
                nc.vector.tensor_copy(
                    kt[:, ms * S: ms * S + S], ps[:])

            self.poolid += 1
            with tc.tile_pool(name=f"attn_pp{self.poolid}", bufs=3,
                              space="PSUM") as pp, \
                 tc.tile_pool(name=f"attn_pv{self.poolid}", bufs=2,
                              space="PSUM") as pv:
                # slab-interleaved Q/K emission: head pair j's scores
                # need only qt/kt slab j, so alternating Q/K slabs lets
                # the first scores (and exp) start after one slab pair
                # instead of after both full projections
                wqr = wq[:].rearrange("p (s o) -> p s o", s=KS)
                wkr = wk[:].rearrange("p (s o) -> p s o", s=KS)
                qxr = q8[:].rearrange("p (s t) -> p s t", s=KS)
                kxr = kv8[:].rearrange("p (s t) -> p s t", s=KS)
                for ms in range(KS):
                    for wr, xr, ev in ((wqr, qxr, ev_q),
                                       (wkr, kxr, ev_k)):
                        ps = pp.tile([P, S], F32, tag="proj", name="proj")
                        for win in range(S // 256):
                            o = ps[:, win * 256: win * 256 + 256]
                            for kp in range(2):
                                nc.tensor.matmul(
                                    o,
                                    wr[:, 2 * kp: 2 * kp + 2,
                                       ms * P: ms * P + P],
                                    xr[:, 2 * kp: 2 * kp + 2,
                                       win * 256: win * 256 + 256],
                                    start=(kp == 0), stop=(kp == 1),
                                    perf_mode=PM.DoubleRow)
                        ev(ps, ms)

                # V: Form A (x^T as lhsT) -> token-major v [tok, feat],
                # strided into per-head 65-wide blocks, col 64 = 1.
                v4 = vt[:].rearrange("p (m c) -> p m c", m=MT)
                nc.gpsimd.memset(
                    v4[:, :, 0:NH * 65]
                    .rearrange("p m (h c) -> p m h c", h=NH)
                    [:, :, :, 64:65], 1.0)
                xr = kv8[:].rearrange("p (s t) -> p s t", s=KS)
                wvr = wv[:].rearrange("p (s o) -> p s o", s=KS)
                for mt in range(MT):
                    ps = pv.tile([P, 512], F32, tag="vproj", name="vproj")
                    for fw in range(2):
                        o = ps[:, fw * 256: fw * 256 + 256]
                        for kp in range(2):
                            nc.tensor.matmul(
                                o,
                                xr[:, 2 * kp: 2 * kp + 2, mt * P: mt * P + P],
                                wvr[:, 2 * kp: 2 * kp + 2,
                                    fw * 256: fw * 256 + 256],
                                start=(kp == 0), stop=(kp == 1),
                                perf_mode=PM.DoubleRow)
                    nc.scalar.copy(
                        v4[:, mt, 0:NH * 65]
                        .rearrange("p (h c) -> p h c", h=NH)[:, :, 0:64],
                        ps[:].rearrange("p (h d) -> p h d", h=NH))

            self.poolid += 1
            scav = ExitStack()
            sp = scav.enter_context(
                tc.tile_pool(name=f"attn_sc{self.poolid}", bufs=2,
                             space="PSUM"))
            avp = scav.enter_context(
                tc.tile_pool(name=f"attn_av{self.poolid}", bufs=4,
                             space="PSUM"))

            v4 = vt[:].rearrange("p (m c) -> p m c", m=MT)
            sexp = self.scale / (WS * WS)
            # head pairs outer: each pair's softmax rowsums are
            # reciprocal'd + broadcast + applied as soon as the pair is
            # done, overlapping the remaining pairs' scores/exp/AV.
            NT = 2
            for j in range(NH // 2):
                hs = (2 * j, 2 * j + 1)
                # per-pair staging across both qn halves: head h%2==0 in
                # partitions 0-63, h%2==1 in 64-127 (TensorTensor needs
                # matching operand base partitions)
                tmp = up.tile([P, S], BF, tag="avtmp", name="avtmp",
                              bufs=2)
                st = {h: up.tile([1, S], F32, tag="rstage",
                                 name=f"rstage{h % 2}", bufs=4)
                      for h in hs}
                for qn in range(NT):
                    av = {(h, w): avp.tile([65, 256], F32, tag="av",
                                           name=f"av{h}_{w}")
                          for h in hs for w in range(2)}
                    for wave in range(MT // 2):
                        sc = {h: sp.tile([P, 1024], F32, tag="sc",
                                         name=f"sc{h}") for h in hs}
                        for i in range(2):
                            mt = wave * 2 + i
                            for h in hs:
                                bp = (h % 2) * 64
                                sl = h // 2
                                nc.tensor.matmul(
                                    sc[h][:, i * 512: i * 512 + 512],
                                    kt[bp: bp + 64,
                                       sl * S + mt * P: sl * S + mt * P + P],
                                    qt[bp: bp + 64,
                                       sl * S + qn * 512: sl * S + qn * 512 + 512],
                                    start=True, stop=True)
                        ut = {h: up.tile([P, 1024], F8, tag="u",
                                         name=f"u{h}") for h in hs}
                        for h in hs:
                            nc.scalar.activation(
                                ut[h][:], sc[h][:], AF.Exp, scale=sexp,
                                bias=self.lnc_c[:, 0:1])
                        for h in hs:
                            utr = ut[h][:].rearrange("p (i t) -> p i t", i=2)
                            for win in range(2):
                                nc.tensor.matmul(
                                    av[h, win][:],
                                    v4[:, 2 * wave: 2 * wave + 2,
                                       h * 65: h * 65 + 65],
                                    utr[:, :, win * 256: win * 256 + 256],
                                    start=(wave == 0), stop=(wave == 3),
                                    perf_mode=PM.DoubleRow)
                    for h in hs:
                        bp = (h % 2) * 64
                        # unnormalized o^T and rowsum -> bf16 staging
                        for win in range(2):
                            qo = qn * 512 + win * 256
                            nc.vector.tensor_copy(
                                tmp[bp: bp + 64, qo: qo + 256],
                                av[h, win][0:64, :])
                            nc.vector.tensor_copy(
                                st[h][:, qo: qo + 256],
                                av[h, win][64:65, :])
                for h in hs:
                    bp = (h % 2) * 64
                    sl = h // 2
                    rr = up.tile([1, S], BF, tag="rrec", name="rrec")
                    with nc.allow_low_precision(
                            reason="1/rowsum to bf16 is plenty"):
                        nc.vector.reciprocal(rr[:], st[h][:])
                    # full-128 broadcast: HW ucode mishandles
                    # non-zero destination base partitions
                    rbt = ap.tile([P, S], BF, tag="rbh",
                                  name=f"rbh{h % 2}", bufs=4)
                    nc.gpsimd.partition_broadcast(rbt[:], rr[:])
                    nc.vector.tensor_tensor(
                        ou[bp: bp + 64, sl * S: sl * S + S],
                        tmp[bp: bp + 64, :], rbt[bp: bp + 64, :],
                        op=ALU.mult)
            scav.close()
        return ou

    def attention_finish(self, ou8, wo, evict_out, bufs=2):
        """fp8 out projection of a finished attention core."""
        self.proj_f8(wo, ou8, E, E, evict_out, bufs=bufs)

    # ---------- layernorm ----------

    def layer_norm(self, t32, gam, bet, out32, out8, out16=None,
                   out_dma=None, bf_in=False, bet32=None, hi_fi=False):
        """LN over features (partition axis) of t32 [128, KS*S].
        Stats come from a bf16 copy (ones-matmul over partitions); the
        normalize interior runs in bf16 (or fp32 when hi_fi, for the
        final output norm); the fp32 output (if requested) applies
        gamma/bet32 from the interior.  Optional fp8 side output;
        out_dma streams the fp32 output to DRAM per slab.
        bf_in: t32 is already bf16 (skip the cast)."""
        nc = self.nc
        if bet32 is None:
            bet32 = bet
        self.poolid += 1
        with self.tc.tile_pool(name=f"lnsb{self.poolid}", bufs=1) as lnp:
            if bf_in:
                t16 = t32
            else:
                t16 = lnp.tile([P, KS * S], BF, tag="ln_t16")
                for nt in range(2):
                    for k in range(KS):
                        sl = slice(k * S + nt * 512, k * S + nt * 512 + 512)
                        nc.scalar.copy(t16[:, sl], t32[:, sl])
            mu = lnp.tile([1, S], F32, tag="ln_mu", name="ln_mu")
            var = lnp.tile([1, S], F32, tag="ln_row", name="ln_var",
                           bufs=2)
            self.poolid += 1
            with self.tc.tile_pool(name=f"lnp{self.poolid}", bufs=2,
                                   space="PSUM") as sp1:
                for nt in range(2):
                    pmu = sp1.tile([1, 512], F32, tag="ln_stat", name="pmu")
                    psq = sp1.tile([1, 512], F32, tag="ln_stat", name="psq")
                    for k in range(KS):
                        sl = slice(k * S + nt * 512, k * S + nt * 512 + 512)
                        tsq = lnp.tile([P, 512], BF, tag="ln_tsq",
                                       name="ln_tsq", bufs=2)
                        nc.vector.tensor_tensor(tsq[:], t16[:, sl],
                                                t16[:, sl], op=ALU.mult)
                        nc.tensor.matmul(
                            pmu[:], self.ones_mean[:, 0:1], t16[:, sl],
                            start=(k == 0), stop=(k == KS - 1))
                        nc.tensor.matmul(
                            psq[:], self.ones_mean[:, 0:1], tsq[:],
                            start=(k == 0), stop=(k == KS - 1))
                    osl = slice(nt * 512, nt * 512 + 512)
                    nc.vector.tensor_copy(mu[:, osl], pmu[:])
                    mu2 = lnp.tile([1, 512], F32, tag="ln_mu2", name="ln_mu2")
                    nc.vector.tensor_tensor(mu2[:], mu[:, osl], mu[:, osl],
                                            op=ALU.mult)
                    nc.vector.tensor_tensor(var[:, osl], psq[:], mu2[:],
                                            op=ALU.subtract)
            # rstd = exp(-0.5*ln(var+eps)); rows, broadcasts and the
            # normalize interior all split by token half so consumers
            # (cross projections, FFN, output DMA) start at half-time
            idt = F32 if hi_fi else BF
            lnv = lnp.tile([1, S], F32, tag="ln_row", name="ln_lnv",
                           bufs=2)
            rstd = lnp.tile([1, S], F32, tag="ln_row", name="ln_rstd",
                            bufs=2)
            if not hi_fi:
                murow = lnp.tile([1, S], BF, tag="ln_mu16")
                rsrow = lnp.tile([1, S], BF, tag="ln_rstd16")
            mub = lnp.tile([P, S], idt, tag="ln_mub")
            rstdb = lnp.tile([P, S], idt, tag="ln_rstdb")
            nsrc = t32 if hi_fi else t16
            for nt in range(2):
                osl = slice(nt * 512, nt * 512 + 512)
                nc.scalar.activation(lnv[:, osl], var[:, osl], AF.Ln,
                                     bias=self.eps_c[:, 0:1])
                nc.scalar.activation(rstd[:, osl], lnv[:, osl], AF.Exp,
                                     scale=-0.5)
                if hi_fi:
                    mr, rr_ = mu, rstd
                else:
                    nc.vector.tensor_copy(murow[:, osl], mu[:, osl])
                    nc.vector.tensor_copy(rsrow[:, osl], rstd[:, osl])
                    mr, rr_ = murow, rsrow
                nc.gpsimd.partition_broadcast(mub[:, osl], mr[:, osl])
                nc.gpsimd.partition_broadcast(rstdb[:, osl], rr_[:, osl])
                for k in range(KS):
                    sl = slice(k * S + nt * 512, k * S + nt * 512 + 512)
                    w = lnp.tile([P, 512], idt, tag="ln_w", name="ln_w",
                                 bufs=3)
                    nc.vector.tensor_tensor(w[:], nsrc[:, sl], mub[:, osl],
                                            op=ALU.subtract)
                    nc.vector.tensor_tensor(w[:], w[:], rstdb[:, osl],
                                            op=ALU.mult)
                    if out32 is not None:
                        nc.vector.tensor_scalar(
                            out32[:, sl], w[:], gam[:, k: k + 1],
                            bet32[:, k: k + 1], op0=ALU.mult, op1=ALU.add)
                    if out16 is not None:
                        nc.vector.tensor_scalar(
                            out16[:, sl], w[:], gam[:, k: k + 1],
                            bet[:, k: k + 1], op0=ALU.mult, op1=ALU.add)
                    if out8 is not None:
                        nc.gpsimd.tensor_scalar(
                            out8[:, sl], w[:], gam[:, k: k + 1],
                            bet[:, k: k + 1], op0=ALU.mult, op1=ALU.add)
                    if out32 is not None and out_dma is not None:
                        nc.sync.dma_start(
                            out_dma.rearrange("(s p) t -> p s t", p=P)
                            [:, k, nt * 512: nt * 512 + 512],
                            out32[:, sl])

    # ---------- main ----------

    def run(self, xo32, xt16, xo8, xt8, attw, f1w, f1b, f2w, gw, gbd,
            nrm, out_t):
        nc, tc, ctx = self.nc, self.tc, self.ctx

        const = ctx.enter_context(tc.tile_pool(name="const", bufs=1))

        self.ones_mean = const.tile([P, 1], BF)
        nc.vector.memset(self.ones_mean[:], 1.0 / E)
        self.eps_c = const.tile([1, 1], F32)
        nc.vector.memset(self.eps_c[:], EPS)
        self.lnc_c = const.tile([P, 1], F32)
        nc.vector.memset(self.lnc_c[:], LN_C)
        # ---- weight prefetch: set 'a' first, then the stage-1 inputs
        # (unblocking the first projections ASAP), then the rest ----
        wp = ctx.enter_context(tc.tile_pool(name="wp_all", bufs=1))
        act = ctx.enter_context(tc.tile_pool(name="acts", bufs=1))
        oup = ctx.enter_context(tc.tile_pool(name="oup", bufs=1))

        W = {}

        def load_set(tag):
            for m in ("qw", "kw", "vw", "ow"):
                W[tag + m] = self.load_w8(wp, attw[tag + m], E, E, tag + m)
            W[tag + "qb"] = self.load_vec(wp, attw[tag + "qb"], KS,
                                          tag + "qb")

        load_set("a")

        # ---- stage 1: self-attention + LN for both streams ----
        s1 = ExitStack()
        pools = {st: s1.enter_context(tc.tile_pool(name="sb_" + st,
                                                   bufs=1))
                 for st in ("own", "oth")}
        s1x = ExitStack()
        x8p = s1x.enter_context(tc.tile_pool(name="s1x", bufs=1))
        xin = {}
        # fp8 matmul inputs first (startup-critical); the fat fp32/bf16
        # residual streams aren't read until the first out-projection
        # (~60us in) so they queue after the weight sets
        for st, (x32d, x8d) in (("own", (xo32, xo8)),
                                ("oth", (xt16, xt8))):
            x8 = x8p.tile([P, KS * S], F8, tag="x8", name="x8" + st,
                          bufs=2)
            nc.sync.dma_start(
                x8[:].rearrange("p (s t) -> p s t", s=KS),
                x8d.rearrange("(s p) t -> p s t", p=P))
            xin[st] = [pools[st], x8, None]

        load_set("b")
        load_set("c")
        for st, x32d in (("own", xo32), ("oth", xt16)):
            xdt = F32 if st == "own" else BF
            x32 = pools[st].tile([P, KS * S], xdt, tag="x32", name="x32")
            nc.sync.dma_start(
                x32[:].rearrange("p (s t) -> p s t", s=KS),
                x32d.rearrange("(s p) t -> p s t", p=P))
            xin[st][2] = x32
        W["cob"] = self.load_vec(wp, attw["cob"], KS, "cob")
        # slab stride padded to 16B: dual-fp8 Ldweights pair-stride rule
        gw_sb = wp.tile([P, 8 * 16], F8, tag="gw")
        nc.sync.dma_start(
            gw_sb[:].rearrange("p (s o) -> p s o", s=8)[:, :, 0:2],
            gw.rearrange("(s p) o -> p s o", p=P))
        # norm params / gate consts aren't needed until the first LN
        # (~100us in) — load them after the startup-critical DMAs
        self.gbdneg = const.tile([1, 1], F32)
        nc.sync.dma_start(self.gbdneg[:], gbd[:])
        nc.vector.tensor_scalar(self.gbdneg[:], self.gbdneg[:], -1.0, None,
                                op0=ALU.mult)
        gam = {t: self.load_vec(const, nrm[t + "g"], KS, name=t + "g")
               for t in ("nao", "nat", "nb", "nc")}
        bet = {t: self.load_vec(const, nrm[t + "b"], KS, name=t + "b")
               for t in ("nao", "nat", "nb", "nc")}
        bet["nb2"] = self.load_vec(const, nrm["nbb2"], KS, name="nb2")

        ou1 = {}
        for st, wtag in (("own", "a"), ("oth", "b")):
            sbp, x8, x32 = xin[st]
            ou1[st] = self.attention_core(
                st, x8, x8, W[wtag + "qw"], W[wtag + "kw"],
                W[wtag + "vw"], W[wtag + "qb"], oup)
        s1x.close()

        y16 = None
        y8 = {}
        for st, (wtag, ntag) in (("own", ("a", "nao")),
                                 ("oth", ("b", "nat"))):
            sbp, x8, x32 = xin[st]
            t1 = x32  # residual accumulates in place over the input
            # residual is pre-scaled x16 with ob folded in on the host;
            # psum is 256*(o@ow), so t1 = 16*(true t1).  LN is
            # scale-invariant.

            def ev_out(ps, ms, _t1=t1):
                sl = slice(ms * S, ms * S + S)
                nc.vector.scalar_tensor_tensor(
                    _t1[:, sl], ps[:], 1.0 / WS, _t1[:, sl],
                    op0=ALU.mult, op1=ALU.add)

            self.attention_finish(ou1[st], W[wtag + "ow"], ev_out)
            if st == "own":
                y16 = act.tile([P, KS * S], BF, tag="a16",
                               name="yo16", bufs=2)
                y8[st] = act.tile([P, KS * S], F8, tag="a8",
                                  name="yo8", bufs=3)
                self.layer_norm(t1, gam[ntag], bet[ntag], None, y8[st],
                                out16=y16)
            else:
                y8[st] = act.tile([P, KS * S], F8, tag="a8",
                                  name="yt8", bufs=3)
                self.layer_norm(t1, gam[ntag], bet[ntag], None, y8[st],
                                bf_in=True)
        s1.close()

        # ---- stage 2: cross attention ----
        # FFN weights load here: early enough to overlap, after the
        # stage-1 SBUF peak has passed.
        wpf = ctx.enter_context(tc.tile_pool(name="wp_ffn", bufs=1))
        w1 = self.load_w8(wpf, f1w, E, HID, "w1", dty=BF)
        b1 = self.load_vec(wpf, f1b, HKS, "b1")
        w2 = self.load_w8(wpf, f2w, HID, E, "w2")

        cross16 = act.tile([P, KS * S], BF, tag="a16", bufs=2)
        cross8 = act.tile([P, KS * S], F8, tag="a8", bufs=3)
        with ExitStack() as sctx:
            sbp = sctx.enter_context(tc.tile_pool(name="sb_c", bufs=1))
            ob = W["cob"]

            ouc = self.attention_core(
                "cross", y8["own"], y8["oth"], W["cqw"], W["ckw"],
                W["cvw"], W["cqb"], oup)

            def ev_cross(ps, ms, _ob=ob):
                sl = slice(ms * S, ms * S + S)
                nc.vector.tensor_scalar(
                    cross16[:, sl], ps[:], 1.0 / (WS * WS),
                    _ob[:, ms: ms + 1], op0=ALU.mult, op1=ALU.add)
                nc.gpsimd.tensor_copy(cross8[:, sl], cross16[:, sl])

            self.attention_finish(ouc, W["cow"], ev_cross, bufs=4)

        # ---- stage 3: gate + merge + LN_b ----
        with ExitStack() as sctx:
            sbp = sctx.enter_context(tc.tile_pool(name="sb_g", bufs=1))
            g0row = sbp.tile([1, S], BF, tag="g0")
            gwr = gw_sb[:].rearrange("p (s o) -> p s o", s=8)  # o padded 16
            self.poolid += 1
            gp = sctx.enter_context(tc.tile_pool(
                name=f"gp{self.poolid}", bufs=2, space="PSUM"))
            srcs = (y8["own"], cross8)
            for nt in range(2):
                l0 = gp.tile([1, 512], F32, tag="gl", name="gl0")
                l1 = gp.tile([1, 512], F32, tag="gl", name="gl1")
                for half in range(2):  # 0: own slabs 0-3, 1: cross 4-7
                    src = srcs[half]
                    xr = src[:].rearrange("p (s t) -> p s t", s=KS)
                    for kp in range(2):
                        for col, l in ((0, l0), (1, l1)):
                            nc.tensor.matmul(
                                l[:],
                                gwr[:, half * 4 + 2 * kp:
                                    half * 4 + 2 * kp + 2, col: col + 1],
                                xr[:, 2 * kp: 2 * kp + 2,
                                   nt * 512: nt * 512 + 512],
                                start=(half == 0 and kp == 0),
                                stop=(half == 1 and kp == 1),
                                perf_mode=PM.DoubleRow)
                l0s = sbp.tile([1, 512], F32, tag="gl0s", name="gl0s")
                nc.vector.tensor_copy(l0s[:], l0[:])
                d = sbp.tile([1, 512], F32, tag="gd", name="gd")
                nc.vector.tensor_tensor(d[:], l1[:], l0s[:],
                                        op=ALU.subtract)
                # g0 = sigmoid(l0-l1+gbd) = 1/(1+exp(l1-l0-gbd))
                eneg = sbp.tile([1, 512], F32, tag="ge", name="ge")
                nc.scalar.activation(eneg[:], d[:], AF.Exp,
                                     scale=1.0 / WS,
                                     bias=self.gbdneg[:, 0:1])
                den = sbp.tile([1, 512], F32, tag="gden", name="gden")
                nc.vector.tensor_scalar(den[:], eneg[:], 1.0, None,
                                        op0=ALU.add)
                with nc.allow_low_precision(
                        reason="gate weight to bf16 is plenty"):
                    nc.vector.reciprocal(
                        g0row[:, nt * 512: nt * 512 + 512], den[:])
            g0b = sbp.tile([P, S], BF, tag="g0b")
            nc.gpsimd.partition_broadcast(g0b[:], g0row[:])
            t2 = sbp.tile([P, KS * S], BF, tag="t2")
            for k in range(KS):
                sl = slice(k * S, k * S + S)
                w = sbp.tile([P, S], BF, tag="gs", name="gs", bufs=2)
                nc.vector.tensor_tensor(w[:], y16[:, sl],
                                        cross16[:, sl], op=ALU.subtract)
                nc.vector.tensor_tensor(w[:], w[:], g0b[:], op=ALU.mult)
                nc.vector.tensor_tensor(t2[:, sl], w[:], cross16[:, sl],
                                        op=ALU.add)
            z32 = act.tile([P, KS * S], F32, tag="a32", bufs=2)
            z16 = act.tile([P, KS * S], BF, tag="a16", bufs=2)
            # z32 carries beta+f2b (so the FFN residual add needs no
            # separate bias); z16 (the FFN input) uses the true beta.
            self.layer_norm(t2, gam["nb"], bet["nb"], z32, z16,
                            bet32=bet["nb2"], bf_in=True)

        # ---- stage 4: FFN (fp8) + LN_c + output ----
        with ExitStack() as sctx:
            sbp = sctx.enter_context(tc.tile_pool(name="sb_f", bufs=1))
            t3 = z32  # FFN residual accumulates in place over z32
            with ExitStack() as fctx:
                hp = fctx.enter_context(tc.tile_pool(name="hp_f", bufs=1))
                h8 = hp.tile([P, HKS * S], F8, tag="h8")

                def ev_gelu(ps, ms):
                    nc.scalar.activation(
                        h8[:, ms * S: ms * S + S],
                        ps[:], AF.Gelu, bias=b1[:, ms: ms + 1])

                self.proj_bf(w1, z16, E, HID, ev_gelu)

                def ev_f2(ps, ms, half):
                    sl = slice(ms * S + half * 512,
                               ms * S + half * 512 + 512)
                    nc.vector.scalar_tensor_tensor(
                        t3[:, sl], ps[:, half * 512: half * 512 + 512],
                        1.0 / WS, z32[:, sl], op0=ALU.mult, op1=ALU.add)

                self.proj_f8(w2, h8, HID, E, ev_f2, halves=True)

            out32 = sbp.tile([P, KS * S], F32, tag="out32")
            self.layer_norm(t3, gam["nc"], bet["nc"], out32, None,
                            out_dma=out_t, hi_fi=True)


_NC_CACHE = {}


def _get_nc(scale):
    key = round(float(scale), 12)
    if key not in _NC_CACHE:
        _NC_CACHE[key] = _build_nc(scale)
    return _NC_CACHE[key]


def _prep_in_maps(inputs):
    """Slice/transform the full inputs into 8 per-core input dicts."""
    f32 = np.float32
    body = np.asarray(inputs["body_feats"], f32)
    limb = np.asarray(inputs["limb_feats"], f32)
    qw = np.asarray(inputs["attn_qw"], f32)
    qb = np.asarray(inputs["attn_qb"], f32)
    kw = np.asarray(inputs["attn_kw"], f32)
    vw = np.asarray(inputs["attn_vw"], f32)
    vb = np.asarray(inputs["attn_vb"], f32)
    ow = np.asarray(inputs["attn_ow"], f32)
    ob = np.asarray(inputs["attn_ob"], f32)
    f1w = np.asarray(inputs["ffn_w1"], f32)
    f1b = np.asarray(inputs["ffn_b1"], f32)
    f2w = np.asarray(inputs["ffn_w2"], f32)
    f2b = np.asarray(inputs["ffn_b2"], f32)
    ns = np.asarray(inputs["norm_scale"], f32)
    nb = np.asarray(inputs["norm_bias"], f32)
    gw = np.asarray(inputs["gate_w"], f32)
    gb = np.asarray(inputs["gate_b"], f32)

    feats = [body, limb]
    ob_eff = [ob[i] + vb[i] @ ow[i] for i in range(4)]
    gbd = np.array([[gb[0] - gb[1]]], f32)
    ln_a = [0, 3]
    ln_c = [2, 5]

    in_maps = []
    for c in range(8):
        b, s = c // 2, c % 2
        o = s          # own stream / self-attn set
        t = 1 - s      # other stream
        cr = 2 + s     # cross-attn set
        xoT = np.ascontiguousarray(feats[o][b].T)
        xtT = np.ascontiguousarray(feats[t][b].T)
        m = {
            # residual streams pre-scaled x16 with the effective out-proj
            # bias folded in (the stage-1 evict adds them to 256x psums
            # scaled by 1/16; LN is scale-invariant)
            "xo32": WS * (xoT + ob_eff[o][:, None]),
            "xt16": (WS * (xtT + ob_eff[t][:, None])).astype(BF16),
            "xo8": xoT.astype(F8NP),
            "xt8": xtT.astype(F8NP),
            "f1w": f1w[s].astype(BF16), "f1b": f1b[s],
            "f2w": (WS * f2w[s]).astype(F8NP), "f2b": f2b[s],
            "gw": (WS * gw).astype(F8NP), "gbd": gbd,
            "naog": ns[ln_a[o]], "naob": nb[ln_a[o]],
            "natg": ns[ln_a[t]], "natb": nb[ln_a[t]],
            "nbg": ns[1], "nbb": nb[1],
            "nbb2": nb[1] + f2b[s],
            "ncg": ns[ln_c[s]], "ncb": nb[ln_c[s]],
            "cob": ob_eff[cr],
        }
        for tag, i in (("a", o), ("b", t), ("c", cr)):
            m[tag + "qw"] = (WS * qw[i]).astype(F8NP)
            m[tag + "kw"] = (WS * kw[i]).astype(F8NP)
            m[tag + "vw"] = (WS * vw[i]).astype(F8NP)
            m[tag + "ow"] = (WS * ow[i]).astype(F8NP)
            m[tag + "qb"] = WS * qb[i]
        in_maps.append(m)
    return in_maps


def kernel(**inputs):
    temp = float(np.asarray(inputs["temperature"]))
    scale = (D ** -0.5) / temp
    nc = _get_nc(scale)
    in_maps = _prep_in_maps(inputs)
    res = run_bass_kernel_spmd(nc, in_maps, core_ids=list(range(8)))
    body = np.empty((B, S, E), np.float32)
    limb = np.empty((B, S, E), np.float32)
    for c in range(8):
        b, s = c // 2, c % 2
        o = res.results[c]["outT"].T
        (body if s == 0 else limb)[b] = o
    return body, limb


# revision 44
# speedup vs baseline: 1.0303x; 1.0137x over previous
"""Trainium2 Bass kernel for nn_DualAttentionLayer (dense dual-stream
transformer layer: 2x self-attention -> cross-attention -> gated merge ->
FFN, with layernorms).

Sharding: 8 cores = 4 batches x 2 streams. Core c handles batch c//2,
stream c%2 (0=body, 1=limb). Each core redundantly computes BOTH streams'
self-attention+LN stage (so no inter-core communication is needed), then
its own stream's cross-attention, gate, FFN and final norms.

v3: fp8 everywhere on the PE + engine rebalance.
 - All projections (QKV, out-proj, FFN w1/w2, gate) run fp8e4m3 with
   DoubleRow perf mode.  Weights pre-scaled x16 on the host; scale
   factors fold into eviction scales / softmax exp scale / LN betas.
 - Self-attn out-proj biases fold into the host-side residual streams.
 - FFN w2 bias folds into LN_b's fp32-path beta.
 - Scores stay bf16 (K=64 per head).  exp() writes fp8e5 u; softmax
   rowsums via a ones-column in V; per-head batched reciprocal (bf16)
   + gpsimd partition broadcast; normalize applies out-of-place into a
   fp8 ou tile that feeds the fp8 out-proj.
 - LayerNorm stats via bf16 ones-matmul; normalize interior in bf16
   (from the bf16 stats copy) even when a fp32 output is requested;
   squares for variance run on Pool.
 - PSUM eviction tiles widened to [128, 1024] (fewer, larger DVE/ACT
   evictions).
"""

import math
import numpy as np
from contextlib import ExitStack

import concourse.bacc as bacc
import concourse.bass as bass
import concourse.mybir as mybir
import concourse.tile as tile
from concourse.bass_utils import run_bass_kernel_spmd

dt = mybir.dt
AF = mybir.ActivationFunctionType
ALU = mybir.AluOpType
PM = mybir.MatmulPerfMode
BF16 = dt.np(dt.bfloat16)
F8NP = dt.np(dt.float8e4)

B, S, E, NH, D = 4, 1024, 512, 8, 64
HID = 4 * E
P = 128
KS = E // P          # 4 feature slabs of 128
MT = S // P          # 8 token m-tiles of 128
HKS = HID // P       # 16 hidden slabs
EPS = 1e-5
WS = 16.0            # host-side fp8 weight scale
C_EXP = 16.0         # softmax exp output scale (cancels in normalization)
LN_C = math.log(C_EXP)
VB = NH * 65 + 8     # v block stride per k-tile, padded to 528:
                     # dual-fp8 Ldweights needs pair stride % 16 == 0

F32 = dt.float32
BF = dt.bfloat16
F8 = dt.float8e4
F8U = dt.float8e5


def _build_nc(scale: float):
    nc = bacc.Bacc("TRN2", target_bir_lowering=False, debug=False,
                   num_devices=8)

    def din(name, shape, dty=F32):
        return nc.dram_tensor(name, shape, dty, kind="ExternalInput").ap()

    # activations (pre-transposed on host, feature-major [E, S])
    xo32 = din("xo32", [E, S])          # own residual = 16*(x+ob_eff), fp32
    xt16 = din("xt16", [E, S], BF)      # oth residual = 16*(x+ob_eff), bf16
    xo8 = din("xo8", [E, S], F8)        # own, fp8 (matmul rhs)
    xt8 = din("xt8", [E, S], F8)

    # attention weight sets: a = self-own, b = self-other, c = cross
    # all fp8 (x16)
    attw = {}
    for tag in ("a", "b", "c"):
        for m in ("qw", "kw", "vw", "ow"):
            attw[tag + m] = din(tag + m, [E, E], F8)
        attw[tag + "qb"] = din(tag + "qb", [E])     # 16*qb
    attw["cob"] = din("cob", [E])       # cross ob + vb@ow (unscaled)

    f1w = din("f1w", [E, HID], BF)      # bf16 (fp8 here costs too much err)
    f1b = din("f1b", [HID])
    f2w = din("f2w", [HID, E], F8)      # 16*w2
    f2b = din("f2b", [E])               # unused on device (folded in nbg2)
    gw = din("gw", [2 * E, 2], F8)
    gbd = din("gbd", [1, 1])            # gate_b[0] - gate_b[1]

    # norm params: a_own, a_oth (post-self-attn), b (post-gate), c (post-ffn)
    # "nb" has two betas: nbb2 = beta + f2b (for the fp32 residual copy)
    nrm = {}
    for tag in ("nao", "nat", "nb", "nc"):
        nrm[tag + "g"] = din(tag + "g", [E])
        nrm[tag + "b"] = din(tag + "b", [E])
    nrm["nbb2"] = din("nbb2", [E])

    out_t = nc.dram_tensor("outT", [E, S], F32, kind="ExternalOutput").ap()

    with TileKernel(nc, scale) as tk:
        tk.run(xo32, xt16, xo8, xt8, attw, f1w, f1b, f2w, gw, gbd,
               nrm, out_t)

    nc.finalize()
    return nc


class TileKernel:
    def __init__(self, nc, scale):
        self.nc = nc
        self.scale = float(scale)
        self.ctx = ExitStack()
        self.poolid = 0

    def __enter__(self):
        self.tc = self.ctx.enter_context(tile.TileContext(self.nc))
        return self

    def __exit__(self, *a):
        return self.ctx.__exit__(*a)

    # ---------- helpers ----------

    def load_vec(self, pool, dram_ap, n, name=None):
        """Load a [n*128] fp32 vector as [128, n] (slab per column)."""
        t = pool.tile([P, n], F32, tag=name)
        self.nc.sync.dma_start(
            t[:], dram_ap.rearrange("(s p) -> p s", p=P))
        return t

    def load_w8(self, pool, dram_ap, in_dim, out_dim, name=None, dty=F8):
        """Load weight [in,out] as [128, (in/128)*out] slab-major."""
        ks = in_dim // P
        t = pool.tile([P, ks * out_dim], dty, tag=name)
        self.nc.sync.dma_start(
            t[:].rearrange("p (s o) -> p s o", s=ks),
            dram_ap.rearrange("(s p) o -> p s o", p=P))
        return t

    def proj_bf(self, wsb, rhs16, in_dim, out_dim, evict, bufs=4):
        """bf16 Form-B projection: out^T = W^T @ x^T."""
        nc = self.nc
        self.poolid += 1
        with self.tc.tile_pool(name=f"pb{self.poolid}", bufs=bufs,
                               space="PSUM") as pp:
            nks = in_dim // P
            wr = wsb[:].rearrange("p (s o) -> p s o", s=nks)
            xr = rhs16[:].rearrange("p (s t) -> p s t", s=nks)
            for ms in range(out_dim // P):
                ps = pp.tile([P, S], F32, tag="proj", name="proj")
                for win in range(2):
                    o = ps[:, win * 512: win * 512 + 512]
                    for k in range(nks):
                        nc.tensor.matmul(
                            o, wr[:, k, ms * P: ms * P + P],
                            xr[:, k, win * 512: win * 512 + 512],
                            start=(k == 0), stop=(k == nks - 1))
                evict(ps, ms)

    def proj_f8(self, wsb, rhs8, in_dim, out_dim, evict, bufs=4,
                halves=False):
        """out^T[out,tok] = (W^T @ x^T) with fp8 DoubleRow matmuls.
        evict(ps, ms) consumes a [128, S] fp32 PSUM tile (full token
        range); with halves=True, evict(ps, ms, half) gets 512-token
        halves as their accumulation chains finish."""
        nc = self.nc
        self.poolid += 1
        with self.tc.tile_pool(name=f"pp{self.poolid}", bufs=bufs,
                               space="PSUM") as pp:
            self._proj_f8(pp, wsb, rhs8, in_dim, out_dim, evict,
                          halves=halves)

    def _proj_f8(self, pp, wsb, rhs8, in_dim, out_dim, evict,
                 halves=False):
        nc = self.nc
        nks = in_dim // P
        npr = nks // 2
        wr = wsb[:].rearrange("p (s o) -> p s o", s=nks)
        xr = rhs8[:].rearrange("p (s t) -> p s t", s=nks)
        for ms in range(out_dim // P):
            ps = pp.tile([P, S], F32, tag="proj", name="proj")
            for win in range(S // 256):
                o = ps[:, win * 256: win * 256 + 256]
                toff = win * 256
                for kp in range(npr):
                    nc.tensor.matmul(
                        o,
                        wr[:, 2 * kp: 2 * kp + 2, ms * P: ms * P + P],
                        xr[:, 2 * kp: 2 * kp + 2, toff: toff + 256],
                        start=(kp == 0), stop=(kp == npr - 1),
                        perf_mode=PM.DoubleRow)
                if halves and win % 2 == 1:
                    evict(ps, ms, win // 2)
            if not halves:
                evict(ps, ms)

    # ---------- attention ----------

    def attention_core(self, name, q8, kv8, wq, wk, wv, qb, oup,
                       kv_first=False):
        """MHA core: fp8 projections, bf16 scores, fp8 exp, DoubleRow AV,
        streamed softmax normalization.  Returns the normalized per-head
        output ou8 (fp8, tile in caller pool `oup`, = 16x true o).
        The caller runs the fp8 out-projection separately
        (attention_finish) so the next attention's core can overlap this
        one's tail."""
        nc, tc = self.nc, self.tc
        ou = oup.tile([P, KS * S], F8, tag="ou", name="ou_" + name,
                      bufs=2)
        with ExitStack() as actx:
            ap = actx.enter_context(
                tc.tile_pool(name="attc_" + name, bufs=1))
            up = actx.enter_context(
                tc.tile_pool(name="attu_" + name, bufs=3))

            qt = ap.tile([P, KS * S], BF, tag="qT")
            kt = ap.tile([P, KS * S], BF, tag="kT")
            vt = ap.tile([P, MT * VB], F8, tag="vT")

            def ev_q(ps, ms):
                nc.vector.tensor_scalar(
                    qt[:, ms * S: ms * S + S],
                    ps[:], qb[:, ms: ms + 1], None, op0=ALU.add)

            def ev_k(ps, ms):
                nc.vector.tensor_copy(
                    kt[:, ms * S: ms * S + S], ps[:])

            self.poolid += 1
            with tc.tile_pool(name=f"attn_pp{self.poolid}", bufs=3,
                              space="PSUM") as pp, \
                 tc.tile_pool(name=f"attn_pv{self.poolid}", bufs=2,
                              space="PSUM") as pv:
                # slab-interleaved Q/K emission: head pair j's scores
                # need only qt/kt slab j, so alternating Q/K slabs lets
                # the first scores (and exp) start after one slab pair
                # instead of after both full projections
                wqr = wq[:].rearrange("p (s o) -> p s o", s=KS)
                wkr = wk[:].rearrange("p (s o) -> p s o", s=KS)
                qxr = q8[:].rearrange("p (s t) -> p s t", s=KS)
                kxr = kv8[:].rearrange("p (s t) -> p s t", s=KS)
                for ms in range(KS):
                    for wr, xr, ev in ((wqr, qxr, ev_q),
                                       (wkr, kxr, ev_k)):
                        ps = pp.tile([P, S], F32, tag="proj", name="proj")
                        for win in range(S // 256):
                            o = ps[:, win * 256: win * 256 + 256]
                            for kp in range(2):
                                nc.tensor.matmul(
                                    o,
                                    wr[:, 2 * kp: 2 * kp + 2,
                                       ms * P: ms * P + P],
                                    xr[:, 2 * kp: 2 * kp + 2,
                                       win * 256: win * 256 + 256],
                                    start=(kp == 0), stop=(kp == 1),
                                    perf_mode=PM.DoubleRow)
                        ev(ps, ms)

                # V: Form A (x^T as lhsT) -> token-major v [tok, feat],
                # strided into per-head 65-wide blocks, col 64 = 1.
                v4 = vt[:].rearrange("p (m c) -> p m c", m=MT)
                nc.gpsimd.memset(
                    v4[:, :, 0:NH * 65]
                    .rearrange("p m (h c) -> p m h c", h=NH)
                    [:, :, :, 64:65], 1.0)
                xr = kv8[:].rearrange("p (s t) -> p s t", s=KS)
                wvr = wv[:].rearrange("p (s o) -> p s o", s=KS)
                for mt in range(MT):
                    ps = pv.tile([P, 512], F32, tag="vproj", name="vproj")
                    for fw in range(2):
                        o = ps[:, fw * 256: fw * 256 + 256]
                        for kp in range(2):
                            nc.tensor.matmul(
                                o,
                                xr[:, 2 * kp: 2 * kp + 2, mt * P: mt * P + P],
                                wvr[:, 2 * kp: 2 * kp + 2,
                                    fw * 256: fw * 256 + 256],
                                start=(kp == 0), stop=(kp == 1),
                                perf_mode=PM.DoubleRow)
                    nc.scalar.copy(
                        v4[:, mt, 0:NH * 65]
                        .rearrange("p (h c) -> p h c", h=NH)[:, :, 0:64],
                        ps[:].rearrange("p (h d) -> p h d", h=NH))

            self.poolid += 1
            scav = ExitStack()
            sp = scav.enter_context(
                tc.tile_pool(name=f"attn_sc{self.poolid}", bufs=2,
                             space="PSUM"))
            avp = scav.enter_context(
                tc.tile_pool(name=f"attn_av{self.poolid}", bufs=4,
                             space="PSUM"))

            v4 = vt[:].rearrange("p (m c) -> p m c", m=MT)
            sexp = self.scale / (WS * WS)
            # head pairs outer: each pair's softmax rowsums are
            # reciprocal'd + broadcast + applied as soon as the pair is
            # done, overlapping the remaining pairs' scores/exp/AV.
            NT = 2
            for j in range(NH // 2):
                hs = (2 * j, 2 * j + 1)
                # per-pair staging across both qn halves: head h%2==0 in
                # partitions 0-63, h%2==1 in 64-127 (TensorTensor needs
                # matching operand base partitions)
                tmp = up.tile([P, S], BF, tag="avtmp", name="avtmp",
                              bufs=2)
                st = {h: up.tile([1, S], F32, tag="rstage",
                                 name=f"rstage{h % 2}", bufs=4)
                      for h in hs}
                for qn in range(NT):
                    av = {(h, w): avp.tile([65, 256], F32, tag="av",
                                           name=f"av{h}_{w}")
                          for h in hs for w in range(2)}
                    for wave in range(MT // 2):
                        sc = {h: sp.tile([P, 1024], F32, tag="sc",
                                         name=f"sc{h}") for h in hs}
                        for i in range(2):
                            mt = wave * 2 + i
                            for h in hs:
                                bp = (h % 2) * 64
                                sl = h // 2
                                nc.tensor.matmul(
                                    sc[h][:, i * 512: i * 512 + 512],
                                    kt[bp: bp + 64,
                                       sl * S + mt * P: sl * S + mt * P + P],
                                    qt[bp: bp + 64,
                                       sl * S + qn * 512: sl * S + qn * 512 + 512],
                                    start=True, stop=True)
                        ut = {h: up.tile([P, 1024], F8, tag="u",
                                         name=f"u{h}") for h in hs}
                        for h in hs:
                            nc.scalar.activation(
                                ut[h][:], sc[h][:], AF.Exp, scale=sexp,
                                bias=self.lnc_c[:, 0:1])
                        for h in hs:
                            utr = ut[h][:].rearrange("p (i t) -> p i t", i=2)
                            for win in range(2):
                                nc.tensor.matmul(
                                    av[h, win][:],
                                    v4[:, 2 * wave: 2 * wave + 2,
                                       h * 65: h * 65 + 65],
                                    utr[:, :, win * 256: win * 256 + 256],
                                    start=(wave == 0), stop=(wave == 3),
                                    perf_mode=PM.DoubleRow)
                    for h in hs:
                        bp = (h % 2) * 64
                        # unnormalized o^T and rowsum -> bf16 staging
                        for win in range(2):
                            qo = qn * 512 + win * 256
                            nc.vector.tensor_copy(
                                tmp[bp: bp + 64, qo: qo + 256],
                                av[h, win][0:64, :])
                            nc.vector.tensor_copy(
                                st[h][:, qo: qo + 256],
                                av[h, win][64:65, :])
                for h in hs:
                    bp = (h % 2) * 64
                    sl = h // 2
                    rr = up.tile([1, S], BF, tag="rrec", name="rrec")
                    with nc.allow_low_precision(
                            reason="1/rowsum to bf16 is plenty"):
                        nc.vector.reciprocal(rr[:], st[h][:])
                    # full-128 broadcast: HW ucode mishandles
                    # non-zero destination base partitions
                    rbt = ap.tile([P, S], BF, tag="rbh",
                                  name=f"rbh{h % 2}", bufs=4)
                    nc.gpsimd.partition_broadcast(rbt[:], rr[:])
                    nc.vector.tensor_tensor(
                        ou[bp: bp + 64, sl * S: sl * S + S],
                        tmp[bp: bp + 64, :], rbt[bp: bp + 64, :],
                        op=ALU.mult)
            scav.close()
        return ou

    def attention_finish(self, ou8, wo, evict_out, bufs=2):
        """fp8 out projection of a finished attention core."""
        self.proj_f8(wo, ou8, E, E, evict_out, bufs=bufs)

    # ---------- layernorm ----------

    def layer_norm(self, t32, gam, bet, out32, out8, out16=None,
                   out_dma=None, bf_in=False, bet32=None, hi_fi=False):
        """LN over features (partition axis) of t32 [128, KS*S].
        Stats come from a bf16 copy (ones-matmul over partitions); the
        normalize interior runs in bf16 (or fp32 when hi_fi, for the
        final output norm); the fp32 output (if requested) applies
        gamma/bet32 from the interior.  Optional fp8 side output;
        out_dma streams the fp32 output to DRAM per slab.
        bf_in: t32 is already bf16 (skip the cast)."""
        nc = self.nc
        if bet32 is None:
            bet32 = bet
        self.poolid += 1
        with self.tc.tile_pool(name=f"lnsb{self.poolid}", bufs=1) as lnp:
            if bf_in:
                t16 = t32
            else:
                t16 = lnp.tile([P, KS * S], BF, tag="ln_t16")
                for nt in range(2):
                    for k in range(KS):
                        sl = slice(k * S + nt * 512, k * S + nt * 512 + 512)
                        nc.scalar.copy(t16[:, sl], t32[:, sl])
            mu = lnp.tile([1, S], F32, tag="ln_mu", name="ln_mu")
            var = lnp.tile([1, S], F32, tag="ln_row", name="ln_var",
                           bufs=2)
            self.poolid += 1
            with self.tc.tile_pool(name=f"lnp{self.poolid}", bufs=2,
                                   space="PSUM") as sp1:
                for nt in range(2):
                    pmu = sp1.tile([1, 512], F32, tag="ln_stat", name="pmu")
                    psq = sp1.tile([1, 512], F32, tag="ln_stat", name="psq")
                    for k in range(KS):
                        sl = slice(k * S + nt * 512, k * S + nt * 512 + 512)
                        tsq = lnp.tile([P, 512], BF, tag="ln_tsq",
                                       name="ln_tsq", bufs=2)
                        nc.vector.tensor_tensor(tsq[:], t16[:, sl],
                                                t16[:, sl], op=ALU.mult)
                        nc.tensor.matmul(
                            pmu[:], self.ones_mean[:, 0:1], t16[:, sl],
                            start=(k == 0), stop=(k == KS - 1))
                        nc.tensor.matmul(
                            psq[:], self.ones_mean[:, 0:1], tsq[:],
                            start=(k == 0), stop=(k == KS - 1))
                    osl = slice(nt * 512, nt * 512 + 512)
                    nc.vector.tensor_copy(mu[:, osl], pmu[:])
                    mu2 = lnp.tile([1, 512], F32, tag="ln_mu2", name="ln_mu2")
                    nc.vector.tensor_tensor(mu2[:], mu[:, osl], mu[:, osl],
                                            op=ALU.mult)
                    nc.vector.tensor_tensor(var[:, osl], psq[:], mu2[:],
                                            op=ALU.subtract)
            # rstd = exp(-0.5*ln(var+eps)); rows, broadcasts and the
            # normalize interior all split by token half so consumers
            # (cross projections, FFN, output DMA) start at half-time
            idt = F32 if hi_fi else BF
            lnv = lnp.tile([1, S], F32, tag="ln_row", name="ln_lnv",
                           bufs=2)
            rstd = lnp.tile([1, S], F32, tag="ln_row", name="ln_rstd",
                            bufs=2)
            if not hi_fi:
                murow = lnp.tile([1, S], BF, tag="ln_mu16")
                rsrow = lnp.tile([1, S], BF, tag="ln_rstd16")
            mub = lnp.tile([P, S], idt, tag="ln_mub")
            rstdb = lnp.tile([P, S], idt, tag="ln_rstdb")
            nsrc = t32 if hi_fi else t16
            for nt in range(2):
                osl = slice(nt * 512, nt * 512 + 512)
                nc.scalar.activation(lnv[:, osl], var[:, osl], AF.Ln,
                                     bias=self.eps_c[:, 0:1])
                nc.scalar.activation(rstd[:, osl], lnv[:, osl], AF.Exp,
                                     scale=-0.5)
                if hi_fi:
                    mr, rr_ = mu, rstd
                else:
                    nc.vector.tensor_copy(murow[:, osl], mu[:, osl])
                    nc.vector.tensor_copy(rsrow[:, osl], rstd[:, osl])
                    mr, rr_ = murow, rsrow
                nc.gpsimd.partition_broadcast(mub[:, osl], mr[:, osl])
                nc.gpsimd.partition_broadcast(rstdb[:, osl], rr_[:, osl])
                for k in range(KS):
                    sl = slice(k * S + nt * 512, k * S + nt * 512 + 512)
                    w = lnp.tile([P, 512], idt, tag="ln_w", name="ln_w",
                                 bufs=3)
                    nc.vector.tensor_tensor(w[:], nsrc[:, sl], mub[:, osl],
                                            op=ALU.subtract)
                    nc.vector.tensor_tensor(w[:], w[:], rstdb[:, osl],
                                            op=ALU.mult)
                    if out32 is not None:
                        nc.vector.tensor_scalar(
                            out32[:, sl], w[:], gam[:, k: k + 1],
                            bet32[:, k: k + 1], op0=ALU.mult, op1=ALU.add)
                    if out16 is not None:
                        nc.vector.tensor_scalar(
                            out16[:, sl], w[:], gam[:, k: k + 1],
                            bet[:, k: k + 1], op0=ALU.mult, op1=ALU.add)
                    if out8 is not None:
                        nc.gpsimd.tensor_scalar(
                            out8[:, sl], w[:], gam[:, k: k + 1],
                            bet[:, k: k + 1], op0=ALU.mult, op1=ALU.add)
                    if out32 is not None and out_dma is not None:
                        nc.sync.dma_start(
                            out_dma.rearrange("(s p) t -> p s t", p=P)
                            [:, k, nt * 512: nt * 512 + 512],
                            out32[:, sl])

    # ---------- main ----------

    def run(self, xo32, xt16, xo8, xt8, attw, f1w, f1b, f2w, gw, gbd,
            nrm, out_t):
        nc, tc, ctx = self.nc, self.tc, self.ctx

        const = ctx.enter_context(tc.tile_pool(name="const", bufs=1))

        self.ones_mean = const.tile([P, 1], BF)
        nc.vector.memset(self.ones_mean[:], 1.0 / E)
        self.eps_c = const.tile([1, 1], F32)
        nc.vector.memset(self.eps_c[:], EPS)
        self.lnc_c = const.tile([P, 1], F32)
        nc.vector.memset(self.lnc_c[:], LN_C)
        # ---- weight prefetch: set 'a' first, then the stage-1 inputs
        # (unblocking the first projections ASAP), then the rest ----
        wp = ctx.enter_context(tc.tile_pool(name="wp_all", bufs=1))
        act = ctx.enter_context(tc.tile_pool(name="acts", bufs=1))
        oup = ctx.enter_context(tc.tile_pool(name="oup", bufs=1))

        W = {}

        def load_set(tag):
            for m in ("qw", "kw", "vw", "ow"):
                W[tag + m] = self.load_w8(wp, attw[tag + m], E, E, tag + m)
            W[tag + "qb"] = self.load_vec(wp, attw[tag + "qb"], KS,
                                          tag + "qb")

        load_set("a")

        # ---- stage 1: self-attention + LN for both streams ----
        s1 = ExitStack()
        pools = {st: s1.enter_context(tc.tile_pool(name="sb_" + st,
                                                   bufs=1))
                 for st in ("own", "oth")}
        s1x = ExitStack()
        x8p = s1x.enter_context(tc.tile_pool(name="s1x", bufs=1))
        xin = {}
        # fp8 matmul inputs first (startup-critical); the fat fp32/bf16
        # residual streams aren't read until the first out-projection
        # (~60us in) so they queue after the weight sets
        for st, (x32d, x8d) in (("own", (xo32, xo8)),
                                ("oth", (xt16, xt8))):
            x8 = x8p.tile([P, KS * S], F8, tag="x8", name="x8" + st,
                          bufs=2)
            for sp_ in range(2):
                nc.sync.dma_start(
                    x8[:].rearrange("p (s t) -> p s t", s=KS)
                    [:, 2 * sp_: 2 * sp_ + 2, :],
                    x8d.rearrange("(s p) t -> p s t", p=P)
                    [:, 2 * sp_: 2 * sp_ + 2, :])
            xin[st] = [pools[st], x8, None]

        load_set("b")
        load_set("c")
        for st, x32d in (("own", xo32), ("oth", xt16)):
            xdt = F32 if st == "own" else BF
            x32 = pools[st].tile([P, KS * S], xdt, tag="x32", name="x32")
            nc.sync.dma_start(
                x32[:].rearrange("p (s t) -> p s t", s=KS),
                x32d.rearrange("(s p) t -> p s t", p=P))
            xin[st][2] = x32
        W["cob"] = self.load_vec(wp, attw["cob"], KS, "cob")
        # slab stride padded to 16B: dual-fp8 Ldweights pair-stride rule
        gw_sb = wp.tile([P, 8 * 16], F8, tag="gw")
        nc.sync.dma_start(
            gw_sb[:].rearrange("p (s o) -> p s o", s=8)[:, :, 0:2],
            gw.rearrange("(s p) o -> p s o", p=P))
        # norm params / gate consts aren't needed until the first LN
        # (~100us in) — load them after the startup-critical DMAs
        self.gbdneg = const.tile([1, 1], F32)
        nc.sync.dma_start(self.gbdneg[:], gbd[:])
        nc.vector.tensor_scalar(self.gbdneg[:], self.gbdneg[:], -1.0, None,
                                op0=ALU.mult)
        gam = {t: self.load_vec(const, nrm[t + "g"], KS, name=t + "g")
               for t in ("nao", "nat", "nb", "nc")}
        bet = {t: self.load_vec(const, nrm[t + "b"], KS, name=t + "b")
               for t in ("nao", "nat", "nb", "nc")}
        bet["nb2"] = self.load_vec(const, nrm["nbb2"], KS, name="nb2")

        ou1 = {}
        for st, wtag in (("own", "a"), ("oth", "b")):
            sbp, x8, x32 = xin[st]
            ou1[st] = self.attention_core(
                st, x8, x8, W[wtag + "qw"], W[wtag + "kw"],
                W[wtag + "vw"], W[wtag + "qb"], oup)
        s1x.close()

        y16 = None
        y8 = {}
        for st, (wtag, ntag) in (("own", ("a", "nao")),
                                 ("oth", ("b", "nat"))):
            sbp, x8, x32 = xin[st]
            t1 = x32  # residual accumulates in place over the input
            # residual is pre-scaled x16 with ob folded in on the host;
            # psum is 256*(o@ow), so t1 = 16*(true t1).  LN is
            # scale-invariant.

            def ev_out(ps, ms, _t1=t1):
                sl = slice(ms * S, ms * S + S)
                nc.vector.scalar_tensor_tensor(
                    _t1[:, sl], ps[:], 1.0 / WS, _t1[:, sl],
                    op0=ALU.mult, op1=ALU.add)

            self.attention_finish(ou1[st], W[wtag + "ow"], ev_out)
            if st == "own":
                y16 = act.tile([P, KS * S], BF, tag="a16",
                               name="yo16", bufs=2)
                y8[st] = act.tile([P, KS * S], F8, tag="a8",
                                  name="yo8", bufs=3)
                self.layer_norm(t1, gam[ntag], bet[ntag], None, y8[st],
                                out16=y16)
            else:
                y8[st] = act.tile([P, KS * S], F8, tag="a8",
                                  name="yt8", bufs=3)
                self.layer_norm(t1, gam[ntag], bet[ntag], None, y8[st],
                                bf_in=True)
        s1.close()

        # ---- stage 2: cross attention ----
        # FFN weights load here: early enough to overlap, after the
        # stage-1 SBUF peak has passed.
        wpf = ctx.enter_context(tc.tile_pool(name="wp_ffn", bufs=1))
        w1 = self.load_w8(wpf, f1w, E, HID, "w1", dty=BF)
        b1 = self.load_vec(wpf, f1b, HKS, "b1")
        w2 = self.load_w8(wpf, f2w, HID, E, "w2")

        cross16 = act.tile([P, KS * S], BF, tag="a16", bufs=2)
        cross8 = act.tile([P, KS * S], F8, tag="a8", bufs=3)
        with ExitStack() as sctx:
            sbp = sctx.enter_context(tc.tile_pool(name="sb_c", bufs=1))
            ob = W["cob"]

            ouc = self.attention_core(
                "cross", y8["own"], y8["oth"], W["cqw"], W["ckw"],
                W["cvw"], W["cqb"], oup)

            def ev_cross(ps, ms, _ob=ob):
                sl = slice(ms * S, ms * S + S)
                nc.vector.tensor_scalar(
                    cross16[:, sl], ps[:], 1.0 / (WS * WS),
                    _ob[:, ms: ms + 1], op0=ALU.mult, op1=ALU.add)
                nc.gpsimd.tensor_copy(cross8[:, sl], cross16[:, sl])

            self.attention_finish(ouc, W["cow"], ev_cross, bufs=4)

        # ---- stage 3: gate + merge + LN_b ----
        with ExitStack() as sctx:
            sbp = sctx.enter_context(tc.tile_pool(name="sb_g", bufs=1))
            g0row = sbp.tile([1, S], BF, tag="g0")
            gwr = gw_sb[:].rearrange("p (s o) -> p s o", s=8)  # o padded 16
            self.poolid += 1
            gp = sctx.enter_context(tc.tile_pool(
                name=f"gp{self.poolid}", bufs=2, space="PSUM"))
            srcs = (y8["own"], cross8)
            for nt in range(2):
                l0 = gp.tile([1, 512], F32, tag="gl", name="gl0")
                l1 = gp.tile([1, 512], F32, tag="gl", name="gl1")
                for half in range(2):  # 0: own slabs 0-3, 1: cross 4-7
                    src = srcs[half]
                    xr = src[:].rearrange("p (s t) -> p s t", s=KS)
                    for kp in range(2):
                        for col, l in ((0, l0), (1, l1)):
                            nc.tensor.matmul(
                                l[:],
                                gwr[:, half * 4 + 2 * kp:
                                    half * 4 + 2 * kp + 2, col: col + 1],
                                xr[:, 2 * kp: 2 * kp + 2,
                                   nt * 512: nt * 512 + 512],
                                start=(half == 0 and kp == 0),
                                stop=(half == 1 and kp == 1),
                                perf_mode=PM.DoubleRow)
                l0s = sbp.tile([1, 512], F32, tag="gl0s", name="gl0s")
                nc.vector.tensor_copy(l0s[:], l0[:])
                d = sbp.tile([1, 512], F32, tag="gd", name="gd")
                nc.vector.tensor_tensor(d[:], l1[:], l0s[:],
                                        op=ALU.subtract)
                # g0 = sigmoid(l0-l1+gbd) = 1/(1+exp(l1-l0-gbd))
                eneg = sbp.tile([1, 512], F32, tag="ge", name="ge")
                nc.scalar.activation(eneg[:], d[:], AF.Exp,
                                     scale=1.0 / WS,
                                     bias=self.gbdneg[:, 0:1])
                den = sbp.tile([1, 512], F32, tag="gden", name="gden")
                nc.vector.tensor_scalar(den[:], eneg[:], 1.0, None,
                                        op0=ALU.add)
                with nc.allow_low_precision(
                        reason="gate weight to bf16 is plenty"):
                    nc.vector.reciprocal(
                        g0row[:, nt * 512: nt * 512 + 512], den[:])
            g0b = sbp.tile([P, S], BF, tag="g0b")
            t2 = sbp.tile([P, KS * S], BF, tag="t2")
            for nt in range(2):
                osl = slice(nt * 512, nt * 512 + 512)
                nc.gpsimd.partition_broadcast(g0b[:, osl], g0row[:, osl])
                for k in range(KS):
                    sl = slice(k * S + nt * 512, k * S + nt * 512 + 512)
                    w = sbp.tile([P, 512], BF, tag="gs", name="gs",
                                 bufs=3)
                    nc.vector.tensor_tensor(w[:], y16[:, sl],
                                            cross16[:, sl],
                                            op=ALU.subtract)
                    nc.vector.tensor_tensor(w[:], w[:], g0b[:, osl],
                                            op=ALU.mult)
                    nc.vector.tensor_tensor(t2[:, sl], w[:],
                                            cross16[:, sl], op=ALU.add)
            z32 = act.tile([P, KS * S], F32, tag="a32", bufs=2)
            z16 = act.tile([P, KS * S], BF, tag="a16", bufs=2)
            # z32 carries beta+f2b (so the FFN residual add needs no
            # separate bias); z16 (the FFN input) uses the true beta.
            self.layer_norm(t2, gam["nb"], bet["nb"], z32, z16,
                            bet32=bet["nb2"], bf_in=True)

        # ---- stage 4: FFN (fp8) + LN_c + output ----
        with ExitStack() as sctx:
            sbp = sctx.enter_context(tc.tile_pool(name="sb_f", bufs=1))
            t3 = z32  # FFN residual accumulates in place over z32
            with ExitStack() as fctx:
                hp = fctx.enter_context(tc.tile_pool(name="hp_f", bufs=1))
                h8 = hp.tile([P, HKS * S], F8, tag="h8")

                def ev_gelu(ps, ms):
                    nc.scalar.activation(
                        h8[:, ms * S: ms * S + S],
                        ps[:], AF.Gelu, bias=b1[:, ms: ms + 1])

                self.proj_bf(w1, z16, E, HID, ev_gelu)

                def ev_f2(ps, ms, half):
                    sl = slice(ms * S + half * 512,
                               ms * S + half * 512 + 512)
                    nc.vector.scalar_tensor_tensor(
                        t3[:, sl], ps[:, half * 512: half * 512 + 512],
                        1.0 / WS, z32[:, sl], op0=ALU.mult, op1=ALU.add)

                self.proj_f8(w2, h8, HID, E, ev_f2, halves=True)

            out32 = sbp.tile([P, KS * S], F32, tag="out32")
            self.layer_norm(t3, gam["nc"], bet["nc"], out32, None,
                            out_dma=out_t)


_NC_CACHE = {}


def _get_nc(scale):
    key = round(float(scale), 12)
    if key not in _NC_CACHE:
        _NC_CACHE[key] = _build_nc(scale)
    return _NC_CACHE[key]


def _prep_in_maps(inputs):
    """Slice/transform the full inputs into 8 per-core input dicts."""
    f32 = np.float32
    body = np.asarray(inputs["body_feats"], f32)
    limb = np.asarray(inputs["limb_feats"], f32)
    qw = np.asarray(inputs["attn_qw"], f32)
    qb = np.asarray(inputs["attn_qb"], f32)
    kw = np.asarray(inputs["attn_kw"], f32)
    vw = np.asarray(inputs["attn_vw"], f32)
    vb = np.asarray(inputs["attn_vb"], f32)
    ow = np.asarray(inputs["attn_ow"], f32)
    ob = np.asarray(inputs["attn_ob"], f32)
    f1w = np.asarray(inputs["ffn_w1"], f32)
    f1b = np.asarray(inputs["ffn_b1"], f32)
    f2w = np.asarray(inputs["ffn_w2"], f32)
    f2b = np.asarray(inputs["ffn_b2"], f32)
    ns = np.asarray(inputs["norm_scale"], f32)
    nb = np.asarray(inputs["norm_bias"], f32)
    gw = np.asarray(inputs["gate_w"], f32)
    gb = np.asarray(inputs["gate_b"], f32)

    feats = [body, limb]
    ob_eff = [ob[i] + vb[i] @ ow[i] for i in range(4)]
    gbd = np.array([[gb[0] - gb[1]]], f32)
    ln_a = [0, 3]
    ln_c = [2, 5]

    in_maps = []
    for c in range(8):
        b, s = c // 2, c % 2
        o = s          # own stream / self-attn set
        t = 1 - s      # other stream
        cr = 2 + s     # cross-attn set
        xoT = np.ascontiguousarray(feats[o][b].T)
        xtT = np.ascontiguousarray(feats[t][b].T)
        m = {
            # residual streams pre-scaled x16 with the effective out-proj
            # bias folded in (the stage-1 evict adds them to 256x psums
            # scaled by 1/16; LN is scale-invariant)
            "xo32": WS * (xoT + ob_eff[o][:, None]),
            "xt16": (WS * (xtT + ob_eff[t][:, None])).astype(BF16),
            "xo8": xoT.astype(F8NP),
            "xt8": xtT.astype(F8NP),
            "f1w": f1w[s].astype(BF16), "f1b": f1b[s],
            "f2w": (WS * f2w[s]).astype(F8NP), "f2b": f2b[s],
            "gw": (WS * gw).astype(F8NP), "gbd": gbd,
            "naog": ns[ln_a[o]], "naob": nb[ln_a[o]],
            "natg": ns[ln_a[t]], "natb": nb[ln_a[t]],
            "nbg": ns[1], "nbb": nb[1],
            "nbb2": nb[1] + f2b[s],
            "ncg": ns[ln_c[s]], "ncb": nb[ln_c[s]],
            "cob": ob_eff[cr],
        }
        for tag, i in (("a", o), ("b", t), ("c", cr)):
            m[tag + "qw"] = (WS * qw[i]).astype(F8NP)
            m[tag + "kw"] = (WS * kw[i]).astype(F8NP)
            m[tag + "vw"] = (WS * vw[i]).astype(F8NP)
            m[tag + "ow"] = (WS * ow[i]).astype(F8NP)
            m[tag + "qb"] = WS * qb[i]
        in_maps.append(m)
    return in_maps


def kernel(**inputs):
    temp = float(np.asarray(inputs["temperature"]))
    scale = (D ** -0.5) / temp
    nc = _get_nc(scale)
    in_maps = _prep_in_maps(inputs)
    res = run_bass_kernel_spmd(nc, in_maps, core_ids=list(range(8)))
    body = np.empty((B, S, E), np.float32)
    limb = np.empty((B, S, E), np.float32)
    for c in range(8):
        b, s = c // 2, c % 2
        o = res.results[c]["outT"].T
        (body if s == 0 else limb)[b] = o
    return body, limb
